# revision 12
# baseline (speedup 1.0000x reference)
"""Causal self-attention (B=4, T=2048, C=1024, H=16, D=64) on 8 trn2 NeuronCores.

Sharding: core c = (batch b = c//2, head-group g = c%2). Megatron-style within a
batch: each core computes 8 heads' q/k/v (column-parallel) and a row-parallel
partial out-projection. Host sums the two partials per batch and adds the
rank-1 bias term (bo + bv @ wo) -- valid because softmax rows sum to 1, so v's
bias never needs to enter the kernel.

Per-core kernel:
  phase 1 (per 512-wide T chunk): qT,kT = (x@w)^T via fp8e4 DoubleRow matmuls
           (lhsT=w pre-scaled x128 on host, rhs=x^T fp8; 1/128 descale folded
           into the PSUM eviction); v natural via lhsT=x^T-chunk, rhs=wv,
           evicted to fp8e4 with a ones column per head (k-tile pairs padded
           to stride 80 for DoubleRow's step%16 rule).
  phase 2: flash-style streaming attention in S^T orientation:
           S^T[k,q] = kT.T @ qT in bf16 (head pairs in PE row groups 0/64 run
           concurrently via row tiling), P^T = exp(S^T) (ScalarE, writes
           fp8e4 directly; 1/sqrt(D) folded into q), causal masking by an
           in-place [128,128] tril multiply on only the diagonal band;
           O^T accumulated via fp8e4 DoubleRow over k-tile PAIRS
           (lhsT=v pair, rhs=P^T pair); the ones column of v makes PSUM row
           64 the softmax denominator Z for free. The first two k-tiles of
           q-chunk 0 use a bf16 path instead (small-softmax rows have no
           error averaging, fp8 noise there would breach tolerance).
           1/Z via nc.vector.reciprocal, broadcast across partitions with a
           DRAM round-trip (partition-step-0 DMA reads are legal from DRAM).
  phase 3 (per T chunk, overlapped with the next chunk's attention):
           y = O @ wo in bf16 via lhsT=O^T (already the natural layout).
"""
import numpy as np
import ml_dtypes

import concourse.tile as tile
from concourse import bacc, mybir
from concourse.bass_utils import run_bass_kernel_spmd

BF16 = ml_dtypes.bfloat16
FP8 = ml_dtypes.float8_e4m3
F32 = mybir.dt.float32
BT16 = mybir.dt.bfloat16
F8E4 = mybir.dt.float8e4
AF = mybir.ActivationFunctionType
ALU = mybir.AluOpType
DR = mybir.MatmulPerfMode.DoubleRow

B, T, C, H, D = 4, 2048, 1024, 16, 64
G = 2              # head groups (cores per batch)
HL = H // G        # heads per core = 8
HD = HL * D        # local head dims = 512
NP = 4             # head pairs per core
NJQ = T // 512     # q chunks of 512 = 4
NIK = T // 128     # k tiles of 128 = 16
KC = C // 128      # contraction chunks = 8
WS = 128.0         # host pre-scale on fp8 weights

_CACHED = {}


def _build():
    nc = bacc.Bacc("TRN2", debug=False)
    xT = nc.dram_tensor("xT", [C, T], F8E4, kind="ExternalInput").ap()
    xT16 = nc.dram_tensor("xT16", [C, 512], BT16, kind="ExternalInput").ap()
    wq = nc.dram_tensor("wq", [C, HD], F8E4, kind="ExternalInput").ap()
    wk = nc.dram_tensor("wk", [C, HD], F8E4, kind="ExternalInput").ap()
    wv = nc.dram_tensor("wv", [C, HD], F8E4, kind="ExternalInput").ap()
    wq16 = nc.dram_tensor("wq16", [C, HD], BT16, kind="ExternalInput").ap()
    wk16 = nc.dram_tensor("wk16", [C, HD], BT16, kind="ExternalInput").ap()
    wv16 = nc.dram_tensor("wv16", [C, HD], BT16, kind="ExternalInput").ap()
    wo = nc.dram_tensor("wo", [HD, C], BT16, kind="ExternalInput").ap()
    bq = nc.dram_tensor("bq", [128, NP], F32, kind="ExternalInput").ap()
    bk = nc.dram_tensor("bk", [128, NP], F32, kind="ExternalInput").ap()
    masks = nc.dram_tensor("masks", [128, 128], BT16, kind="ExternalInput").ap()
    rcp_dram = nc.dram_tensor("rcp_dram", [NJQ, 8, 512], BT16).ap()
    y = nc.dram_tensor("y", [T, C], F32, kind="ExternalOutput").ap()

    with tile.TileContext(nc) as tc:
        with (
            tc.tile_pool(name="consts", bufs=1) as consts,
            tc.tile_pool(name="xt", bufs=3) as xtp,
            tc.tile_pool(name="qk", bufs=1) as qkp,
            tc.tile_pool(name="vp", bufs=1) as vp,
            tc.tile_pool(name="otp", bufs=1) as otp,
            tc.tile_pool(name="pt", bufs=4) as ptp,
            tc.tile_pool(name="ptb", bufs=2) as ptbp,
            tc.tile_pool(name="zn", bufs=3) as znp,
            tc.tile_pool(name="yst", bufs=4) as ystp,
            tc.tile_pool(name="ps", bufs=2, space="PSUM") as ps,
        ):
            # ---- constants (biases are tiny and gate evictions: load them first) ----
            bq_dma = consts.tile([128, NP], F32, tag="bq_dma")
            bq_sb = consts.tile([128, NP], F32, tag="bq")
            nc.sync.dma_start(bq_dma, bq)
            nc.vector.tensor_copy(bq_sb, bq_dma)
            bk_dma = consts.tile([128, NP], F32, tag="bk_dma")
            bk_sb = consts.tile([128, NP], F32, tag="bk")
            nc.sync.dma_start(bk_dma, bk)
            nc.vector.tensor_copy(bk_sb, bk_dma)
            # bf16 weights + chunk-0 x first: phase1(0) runs in bf16
            wq16_sb = consts.tile([128, KC, HD], BT16, tag="wq16")
            wq16_r = wq16.rearrange("(k p) c -> p k c", p=128)
            xt0 = xtp.tile([128, KC, 512], BT16, tag="xt16", name="xt_pre0")
            x0_r = xT16.rearrange("(k p) t -> p k t", p=128)
            for k in range(KC):
                nc.scalar.dma_start(wq16_sb[:, k, :], wq16_r[:, k, :])
                nc.sync.dma_start(xt0[:, k, :], x0_r[:, k, :])
            wk16_sb = consts.tile([128, KC, HD], BT16, tag="wk16")
            wk16_r = wk16.rearrange("(k p) c -> p k c", p=128)
            for k in range(KC):
                nc.sync.dma_start(wk16_sb[:, k, :], wk16_r[:, k, :])
            masks_dma = consts.tile([128, 128], BT16, tag="masks_dma")
            masks_sb = consts.tile([128, 128], BT16, tag="masks")
            nc.gpsimd.dma_start(masks_dma, masks)
            nc.gpsimd.tensor_copy(masks_sb, masks_dma)
            wv16_sb = consts.tile([128, KC, HD], BT16, tag="wv16")
            nc.gpsimd.dma_start(wv16_sb, wv16.rearrange("(k p) c -> p k c", p=128))
            wq_sb = consts.tile([128, KC, HD], F8E4, tag="wq")
            nc.scalar.dma_start(wq_sb, wq.rearrange("(k p) c -> p k c", p=128))
            wk_sb = consts.tile([128, KC, HD], F8E4, tag="wk")
            nc.sync.dma_start(wk_sb, wk.rearrange("(k p) c -> p k c", p=128))
            wv_sb = consts.tile([128, KC, HD], F8E4, tag="wv")
            nc.gpsimd.dma_start(wv_sb, wv.rearrange("(k p) c -> p k c", p=128))
            wo_sb = consts.tile([128, NP, C], BT16, tag="wo")
            nc.gpsimd.dma_start(wo_sb, wo.rearrange("(t p) c -> p t c", p=128))
            # ---- persistent activations ----
            qT = [qkp.tile([128, T], BT16, tag=f"qT{t}", name=f"qT{t}") for t in range(NP)]
            kT = [qkp.tile([128, T], BT16, tag=f"kT{t}", name=f"kT{t}") for t in range(NP)]
            # v in fp8, [128, k-tile, head, 80] (65 used; 80 keeps DoubleRow pair
            # stride a multiple of 16 bytes)
            v8 = vp.tile([128, NIK, HL, 80], F8E4, tag="v8", name="v8")
            # bf16 copies of k-tiles 0-3 for the jq=0 early-row path
            vb = [vp.tile([128, HL, 65], BT16, tag=f"vb{i}", name=f"vb{i}") for i in range(4)]
            oT = [otp.tile([128, T], BT16, tag=f"oT{t}", name=f"oT{t}") for t in range(NP)]

            def phase1(jt):
                # jt==0 runs entirely in bf16: early attention rows (small
                # softmax support) get no error averaging, and kT tiles 0-3
                # feed every q-chunk, so chunk 0 stays accurate.
                bf = (jt == 0)
                if bf:
                    xt = xt0
                else:
                    xt = xtp.tile([128, KC, 512], F8E4, tag="xt", name=f"xt{jt}")
                    xr = xT[:, jt * 512:(jt + 1) * 512].rearrange("(k p) t -> p k t", p=128)
                    for k in range(KC):
                        (nc.sync if k % 2 == 0 else nc.scalar).dma_start(xt[:, k, :], xr[:, k, :])
                wqs, wks, wvs = (
                    (wq16_sb, wk16_sb, wv16_sb) if bf else (wq_sb, wk_sb, wv_sb)
                )
                qsc = 0.125 if bf else 0.125 / WS
                ksc = 1.0 if bf else 1.0 / WS

                def mm_acc(p, w_sb, tsl):
                    if bf:
                        for k in range(KC):
                            nc.tensor.matmul(
                                p, w_sb[:, k, tsl], xt[:, k, :],
                                start=(k == 0), stop=(k == KC - 1),
                            )
                    else:
                        for k in range(0, KC, 2):
                            nc.tensor.matmul(
                                p, w_sb[:, k:k + 2, tsl], xt[:, k:k + 2, :],
                                start=(k == 0), stop=(k == KC - 2), perf_mode=DR,
                            )

                for t in range(NP):
                    p = ps.tile([128, 512], F32, tag="st", name=f"pq{jt}_{t}")
                    mm_acc(p, wqs, slice(t * 128, (t + 1) * 128))
                    nc.vector.tensor_scalar(
                        qT[t][:, jt * 512:(jt + 1) * 512], p,
                        qsc, bq_sb[:, t:t + 1], ALU.mult, ALU.add,
                    )
                for t in range(NP):
                    p = ps.tile([128, 512], F32, tag="st", name=f"pk{jt}_{t}")
                    mm_acc(p, wks, slice(t * 128, (t + 1) * 128))
                    nc.vector.tensor_scalar(
                        kT[t][:, jt * 512:(jt + 1) * 512], p,
                        ksc, bk_sb[:, t:t + 1], ALU.mult, ALU.add,
                    )
                for s in range(4):
                    ik = jt * 4 + s
                    p = ps.tile([128, 512], F32, tag="st", name=f"pv{ik}")
                    if bf:
                        for k in range(KC):
                            nc.tensor.matmul(
                                p, xt[:, k, s * 128:(s + 1) * 128], wvs[:, k, :],
                                start=(k == 0), stop=(k == KC - 1),
                            )
                    else:
                        for k in range(0, KC, 2):
                            nc.tensor.matmul(
                                p, xt[:, k:k + 2, s * 128:(s + 1) * 128], wvs[:, k:k + 2, :],
                                start=(k == 0), stop=(k == KC - 2), perf_mode=DR,
                            )
                    pg = p.rearrange("p (h c) -> p h c", c=64)
                    if bf:
                        nc.vector.tensor_copy(v8[:, ik, :, 0:64], pg)
                        nc.vector.tensor_copy(vb[ik][:, :, 0:64], pg)
                        nc.vector.memset(vb[ik][:, :, 64:65], 1.0)
                    else:
                        nc.vector.tensor_scalar_mul(v8[:, ik, :, 0:64], pg, 1.0 / WS)
                    nc.vector.memset(v8[:, ik, :, 64:65], 1.0)

            def attention(t, jq):
                nik = 4 * jq + 4
                o_ps = [
                    ps.tile([65, 512], F32, tag="ot", bufs=4, name=f"ops{t}_{jq}_{_h}")
                    for _h in range(2)
                ]
                pt = None
                for ik in range(nik):
                    d = ik - 4 * jq
                    c0 = 128 * d if d > 0 else 0   # first potentially-valid column
                    pr, j = divmod(ik, 2)
                    st = ps.tile([128, 1024], F32, tag="st", name=f"st{t}_{jq}_{ik}")
                    stg = st.rearrange("p (h q) -> p h q", q=512)
                    for hh in range(2):
                        r = slice(hh * 64, hh * 64 + 64)
                        nc.tensor.matmul(
                            stg[:, hh, c0:512],
                            kT[t][r, ik * 128:(ik + 1) * 128],
                            qT[t][r, jq * 512 + c0:(jq + 1) * 512],
                            start=True, stop=True,
                        )
                    if jq == 0:
                        # all 4 tiles of q-chunk 0 take the bf16 path
                        ptb = ptbp.tile([128, 2, 512], BT16, tag="ptb", name=f"ptb{t}_{ik}")
                        nc.scalar.activation(ptb[:, :, c0:512], stg[:, :, c0:512], AF.Exp)
                        for hh in range(2):
                            nc.vector.tensor_mul(
                                ptb[:, hh, c0:c0 + 128], ptb[:, hh, c0:c0 + 128], masks_sb
                            )
                            nc.tensor.matmul(
                                o_ps[hh][:, c0:512],
                                vb[ik][:, 2 * t + hh, :],
                                ptb[:, hh, c0:512],
                                start=(ik == 0), stop=(ik == nik - 1),
                            )
                        continue
                    if j == 0:
                        pt = ptp.tile([128, 2, 2, 512], F8E4, tag="pt", name=f"pt{t}_{jq}_{pr}")
                    if d < 0:
                        nc.scalar.activation(pt[:, j, :, :], stg, AF.Exp)
                    else:
                        nc.scalar.activation(pt[:, j, :, c0:512], stg[:, :, c0:512], AF.Exp)
                        for hh in range(2):
                            nc.vector.tensor_mul(
                                pt[:, j, hh, c0:c0 + 128],
                                pt[:, j, hh, c0:c0 + 128], masks_sb,
                            )
                        if d == 1:
                            nc.vector.memset(pt[:, j, :, 0:128], 0.0)
                        elif d == 3:
                            nc.vector.memset(pt[:, j, :, 256:384], 0.0)
                    if j == 1:
                        cc0 = 256 if d == 3 else 0
                        for hh in range(2):
                            nc.tensor.matmul(
                                o_ps[hh][:, cc0:512],
                                v8[:, 2 * pr:2 * pr + 2, 2 * t + hh, 0:65],
                                pt[:, 0:2, hh, cc0:512],
                                start=(pr == 0 and jq > 0), stop=(ik == nik - 1),
                                perf_mode=DR,
                            )
                # evict Z row + unnormalized O^T, freeing the PSUM accumulators
                out_h = []
                for hh in range(2):
                    ouz = znp.tile([65, 512], F32, tag="ouz", bufs=6, name=f"oz{t}_{jq}_{hh}")
                    nc.vector.tensor_copy(ouz, o_ps[hh])
                    out_h.append(ouz)
                return out_h

            def phase3_m(m):
                    for n in range(2):
                        p = ps.tile([128, 512], F32, tag="st", name=f"py{m}_{n}")
                        for t in range(NP):
                            nc.tensor.matmul(
                                p, oT[t][:, m * 128:(m + 1) * 128],
                                wo_sb[:, t, n * 512:(n + 1) * 512],
                                start=(t == 0), stop=(t == NP - 1),
                            )
                        ys = ystp.tile([128, 512], F32, tag="y", name=f"ys{m}_{n}")
                        nc.vector.tensor_copy(ys, p)
                        nc.gpsimd.dma_start(
                            y[m * 128:(m + 1) * 128, n * 512:(n + 1) * 512], ys
                        )

            def phase3(jq):
                for m in range(4 * jq, 4 * jq + 4):
                    phase3_m(m)

            import concourse.bass as bass_mod

            def normalize(t, jq, evicted, interleave_phase3=False):
                # evicted: [(ouz_h0, ...), (ouz_h1, ...)] for pair t at chunk jq.
                # Pack both heads' Z rows [1,512] as [8,64] each -> one [16,64]
                # reciprocal (64 elems/lane), then broadcast 1/Z via a DRAM
                # round-trip (partition-step-0 DMA reads are legal from DRAM).
                qs2 = slice(jq * 512, (jq + 1) * 512)
                zb = znp.tile([16, 64], F32, tag="zb", bufs=2, name=f"zb{t}_{jq}")
                for hh in range(2):
                    ouz = evicted[hh]
                    nc.sync.dma_start(
                        zb[8 * hh:8 * hh + 8, :],
                        ouz[64:65, :].rearrange("o (p q) -> o p q", p=8),
                    )
                rcp = znp.tile([16, 64], F32, tag="rcpb", bufs=2, name=f"rcp{t}_{jq}")
                nc.vector.reciprocal(rcp, zb)
                rcp16 = znp.tile([16, 64], BT16, tag="rcp16b", bufs=2, name=f"rcp16{t}_{jq}")
                nc.vector.tensor_copy(rcp16, rcp)
                for hh in range(2):
                    nc.sync.dma_start(
                        rcp_dram[jq, 2 * t + hh, :].rearrange("(p q) -> p q", p=8),
                        rcp16[8 * hh:8 * hh + 8, :],
                    )
                tmps = []
                for hh in range(2):
                    ouz = evicted[hh]
                    bc_sb = znp.tile([64, 512], BT16, tag="bc_sb", bufs=3, name=f"bs{t}_{jq}_{hh}")
                    src = rcp_dram[jq, 2 * t + hh, :]
                    bcast = bass_mod.AP(
                        tensor=src.tensor, offset=src.offset,
                        ap=[[0, 64]] + [list(a) for a in src.ap],
                    )
                    nc.sync.dma_start(bc_sb, bcast)
                    tmps.append((ouz, bc_sb))
                if not interleave_phase3:
                    for hh, (ouz, bc_sb) in enumerate(tmps):
                        if hh == 0:
                            nc.vector.tensor_mul(oT[t][0:64, qs2], ouz[0:64, :], bc_sb)
                        else:
                            tmp = znp.tile([64, 512], BT16, tag="tmp_o", bufs=2, name=f"tm{t}_{jq}")
                            nc.vector.tensor_mul(tmp, ouz[0:64, :], bc_sb)
                            nc.gpsimd.dma_start(oT[t][64:128, qs2], tmp)
                else:
                    # last pair of the last chunk: per-128-col muls, phase3
                    # m-chunk follows immediately after its slice is ready
                    for mi in range(4):
                        cs = slice(mi * 128, (mi + 1) * 128)
                        gs = slice(jq * 512 + mi * 128, jq * 512 + (mi + 1) * 128)
                        for hh, (ouz, bc_sb) in enumerate(tmps):
                            if hh == 0:
                                nc.vector.tensor_mul(oT[t][0:64, gs], ouz[0:64, cs], bc_sb[:, cs])
                            else:
                                tmp = znp.tile([64, 128], BT16, tag="tmp_os", bufs=4, name=f"tms{t}_{jq}_{mi}")
                                nc.vector.tensor_mul(tmp, ouz[0:64, cs], bc_sb[:, cs])
                                nc.sync.dma_start(oT[t][64:128, gs], tmp)
                        phase3_m(4 * jq + mi)

            phase1(0)
            pend = []          # (t, jq, evicted) not yet normalized
            for jq in range(NJQ):
                for t in range(NP):
                    ev = attention(t, jq)
                    if jq == 0 and t == 0 and NJQ > 1:
                        phase1(1)
                    if pend:
                        pt_, pjq_, pev_ = pend.pop(0)
                        normalize(pt_, pjq_, pev_)
                        if pt_ == NP - 1:
                            phase3(pjq_)
                    pend.append((t, jq, ev))
                if jq + 2 < NJQ:
                    phase1(jq + 2)
            # tail: all but the last pending entry normally; the last one
            # interleaves its normalization with phase3 m-chunks
            for pt_, pjq_, pev_ in pend[:-1]:
                normalize(pt_, pjq_, pev_)
                if pt_ == NP - 1:
                    phase3(pjq_)
            pt_, pjq_, pev_ = pend[-1]
            normalize(pt_, pjq_, pev_, interleave_phase3=True)

    nc.compile()
    return nc


def _host_prep(x, wq, bq, wk, bk, wv, wo):
    qn = np.arange(128)[None, :]
    kn = np.arange(128)[:, None]
    masks_np = (qn >= kn).astype(BF16)

    per_g = []
    for g in range(G):
        cs = slice(g * HD, (g + 1) * HD)
        per_g.append({
            "wq": np.ascontiguousarray(wq[:, cs] * WS).astype(FP8),
            "wk": np.ascontiguousarray(wk[:, cs] * WS).astype(FP8),
            "wv": np.ascontiguousarray(wv[:, cs] * WS).astype(FP8),
            "wq16": np.ascontiguousarray(wq[:, cs]).astype(BF16),
            "wk16": np.ascontiguousarray(wk[:, cs]).astype(BF16),
            "wv16": np.ascontiguousarray(wv[:, cs]).astype(BF16),
            "wo": np.ascontiguousarray(wo[cs, :]).astype(BF16),
            "bq": np.ascontiguousarray((bq[cs] / 8.0).reshape(NP, 128).T).astype(np.float32),
            "bk": np.ascontiguousarray(bk[cs].reshape(NP, 128).T).astype(np.float32),
            "masks": masks_np,
        })
    in_maps = []
    for c in range(8):
        b, g = divmod(c, G)
        m = dict(per_g[g])
        xt_full = np.ascontiguousarray(x[b].T)
        m["xT"] = xt_full.astype(FP8)
        m["xT16"] = np.ascontiguousarray(xt_full[:, 0:512]).astype(BF16)
        in_maps.append(m)
    return in_maps


def kernel(x, wq, bq, wk, bk, wv, bv, wo, bo):
    x = np.asarray(x, dtype=np.float32)
    wq = np.asarray(wq, dtype=np.float32)
    bq = np.asarray(bq, dtype=np.float32)
    wk = np.asarray(wk, dtype=np.float32)
    bk = np.asarray(bk, dtype=np.float32)
    wv = np.asarray(wv, dtype=np.float32)
    bv = np.asarray(bv, dtype=np.float32)
    wo = np.asarray(wo, dtype=np.float32)
    bo = np.asarray(bo, dtype=np.float32)

    if "nc" not in _CACHED:
        _CACHED["nc"] = _build()
    nc = _CACHED["nc"]

    in_maps = _host_prep(x, wq, bq, wk, bk, wv, wo)
    res = run_bass_kernel_spmd(nc, in_maps, core_ids=list(range(8)))

    const_row = (bo.astype(np.float64) + bv.astype(np.float64) @ wo.astype(np.float64))
    out = np.empty((B, T, C), dtype=np.float32)
    for b in range(B):
        acc = res.results[2 * b]["y"].astype(np.float64)
        acc += res.results[2 * b + 1]["y"]
        acc += const_row[None, :]
        out[b] = acc.astype(np.float32)
    return out


# revision 13
# speedup vs baseline: 1.1356x; 1.1356x over previous
"""Causal self-attention (B=4, T=2048, C=1024, H=16, D=64) on 8 trn2 NeuronCores.

Sharding: core c = (batch b = c//2, head-group g = c%2). Megatron-style within a
batch: each core computes 8 heads' q/k/v (column-parallel) and a row-parallel
partial out-projection. Host sums the two partials per batch and adds the
rank-1 bias term (bo + bv @ wo) -- valid because softmax rows sum to 1, so v's
bias never needs to enter the kernel.

Per-core kernel:
  phase 1 (per 512-wide T chunk): qT,kT = (x@w)^T.  Chunks 1-3 use fp8e4
           DoubleRow matmuls (weights host-prescaled x128 to clear e4m3
           subnormals; x in fp8e4); chunk 0 runs in bf16 because early
           attention rows (tiny softmax support) get no error averaging and
           kT tiles 0-3 feed every q-chunk.  q/k/v all stay x128-scaled in
           SBUF -- the descale is folded into exp's free scale immediate
           (0.125/128^2) and the 1/Z reciprocal (x 1/128), so evictions are
           plain add/copy ops.  v gets a ones column appended per head.
  phase 2: flash-style streaming attention in S^T orientation:
           S^T[k,q] = kT.T @ qT bf16 (head pairs in PE row groups 0/64 run
           concurrently via row tiling), P^T = exp(S^T * 2^-17) on ScalarE;
           causal masking via an in-place [128,128] tril multiply on only
           the diagonal band; O^T accumulated via lhsT=v_tile (stationary),
           rhs=P^T; the ones column of v makes PSUM row 64 the softmax
           denominator Z for free.  1/(128 Z) broadcast across partitions
           with a DRAM round-trip (partition-step-0 DMA reads are legal
           from DRAM).
  phase 3 (per T chunk, overlapped with the next chunk's attention):
           y = O @ wo bf16 via lhsT=O^T (already the natural layout).
"""
import numpy as np
import ml_dtypes

import concourse.tile as tile
from concourse import bacc, mybir
from concourse.bass_utils import run_bass_kernel_spmd

BF16 = ml_dtypes.bfloat16
FP8 = ml_dtypes.float8_e4m3
F32 = mybir.dt.float32
BT16 = mybir.dt.bfloat16
F8E4 = mybir.dt.float8e4
AF = mybir.ActivationFunctionType
ALU = mybir.AluOpType
DR = mybir.MatmulPerfMode.DoubleRow

B, T, C, H, D = 4, 2048, 1024, 16, 64
G = 2              # head groups (cores per batch)
HL = H // G        # heads per core = 8
HD = HL * D        # local head dims = 512
NP = 4             # head pairs per core
NJQ = T // 512     # q chunks of 512 = 4
NIK = T // 128     # k tiles of 128 = 16
KC = C // 128      # contraction chunks = 8
WS = 128.0         # host pre-scale on fp8 weights
SC = 0.125 / (WS * WS)   # exp scale: 1/sqrt(D) and the two x128 descales

_CACHED = {}


def _build():
    nc = bacc.Bacc("TRN2", debug=False)
    xT = nc.dram_tensor("xT", [C, T], F8E4, kind="ExternalInput").ap()
    xT16 = nc.dram_tensor("xT16", [C, 512], BT16, kind="ExternalInput").ap()
    wq = nc.dram_tensor("wq", [C, HD], F8E4, kind="ExternalInput").ap()
    wk = nc.dram_tensor("wk", [C, HD], F8E4, kind="ExternalInput").ap()
    wv = nc.dram_tensor("wv", [C, HD], F8E4, kind="ExternalInput").ap()
    wq16 = nc.dram_tensor("wq16", [C, HD], BT16, kind="ExternalInput").ap()
    wk16 = nc.dram_tensor("wk16", [C, HD], BT16, kind="ExternalInput").ap()
    wv16 = nc.dram_tensor("wv16", [C, HD], BT16, kind="ExternalInput").ap()
    wo = nc.dram_tensor("wo", [HD, C], BT16, kind="ExternalInput").ap()
    bq = nc.dram_tensor("bq", [128, NP], F32, kind="ExternalInput").ap()
    bk = nc.dram_tensor("bk", [128, NP], F32, kind="ExternalInput").ap()
    masks = nc.dram_tensor("masks", [128, 128], BT16, kind="ExternalInput").ap()
    rcp_dram = nc.dram_tensor("rcp_dram", [NJQ, 8, 512], BT16).ap()
    y = nc.dram_tensor("y", [T, C], F32, kind="ExternalOutput").ap()

    with tile.TileContext(nc) as tc:
        with (
            tc.tile_pool(name="consts", bufs=1) as consts,
            tc.tile_pool(name="xt", bufs=3) as xtp,
            tc.tile_pool(name="qk", bufs=1) as qkp,
            tc.tile_pool(name="vp", bufs=1) as vp,
            tc.tile_pool(name="otp", bufs=1) as otp,
            tc.tile_pool(name="pt", bufs=6) as ptp,
            tc.tile_pool(name="zn", bufs=3) as znp,
            tc.tile_pool(name="yst", bufs=4) as ystp,
            tc.tile_pool(name="ps", bufs=2, space="PSUM") as ps,
        ):
            # ---- constants (biases are tiny and gate evictions: load them first) ----
            bq_dma = consts.tile([128, NP], F32, tag="bq_dma")
            bq_sb = consts.tile([128, NP], F32, tag="bq")
            nc.sync.dma_start(bq_dma, bq)
            nc.vector.tensor_copy(bq_sb, bq_dma)
            bk_dma = consts.tile([128, NP], F32, tag="bk_dma")
            bk_sb = consts.tile([128, NP], F32, tag="bk")
            nc.sync.dma_start(bk_dma, bk)
            nc.vector.tensor_copy(bk_sb, bk_dma)
            # bf16 weights + chunk-0 x first: phase1(0) runs in bf16
            wq16_sb = consts.tile([128, KC, HD], BT16, tag="wq16")
            wq16_r = wq16.rearrange("(k p) c -> p k c", p=128)
            xt0 = xtp.tile([128, KC, 512], BT16, tag="xt16", name="xt_pre0")
            x0_r = xT16.rearrange("(k p) t -> p k t", p=128)
            for k in range(KC):
                nc.scalar.dma_start(wq16_sb[:, k, :], wq16_r[:, k, :])
                nc.sync.dma_start(xt0[:, k, :], x0_r[:, k, :])
            wk16_sb = consts.tile([128, KC, HD], BT16, tag="wk16")
            wk16_r = wk16.rearrange("(k p) c -> p k c", p=128)
            for k in range(KC):
                nc.sync.dma_start(wk16_sb[:, k, :], wk16_r[:, k, :])
            masks_dma = consts.tile([128, 128], BT16, tag="masks_dma")
            masks_sb = consts.tile([128, 128], BT16, tag="masks")
            nc.gpsimd.dma_start(masks_dma, masks)
            nc.gpsimd.tensor_copy(masks_sb, masks_dma)
            wv16_sb = consts.tile([128, KC, HD], BT16, tag="wv16")
            nc.gpsimd.dma_start(wv16_sb, wv16.rearrange("(k p) c -> p k c", p=128))
            wq_sb = consts.tile([128, KC, HD], F8E4, tag="wq")
            nc.scalar.dma_start(wq_sb, wq.rearrange("(k p) c -> p k c", p=128))
            wk_sb = consts.tile([128, KC, HD], F8E4, tag="wk")
            nc.sync.dma_start(wk_sb, wk.rearrange("(k p) c -> p k c", p=128))
            wv_sb = consts.tile([128, KC, HD], F8E4, tag="wv")
            nc.gpsimd.dma_start(wv_sb, wv.rearrange("(k p) c -> p k c", p=128))
            wo_sb = consts.tile([128, NP, C], BT16, tag="wo")
            nc.gpsimd.dma_start(wo_sb, wo.rearrange("(t p) c -> p t c", p=128))
            # ---- persistent activations ----
            qT = [qkp.tile([128, T], BT16, tag=f"qT{t}", name=f"qT{t}") for t in range(NP)]
            kT = [qkp.tile([128, T], BT16, tag=f"kT{t}", name=f"kT{t}") for t in range(NP)]
            v_sb = [vp.tile([128, HL * 65], BT16, tag=f"v{i}", name=f"v{i}") for i in range(NIK)]
            oT = [otp.tile([128, T], BT16, tag=f"oT{t}", name=f"oT{t}") for t in range(NP)]

            def phase1(jt):
                # jt==0 runs in bf16: early attention rows (small softmax
                # support) get no error averaging, and kT tiles 0-3 feed
                # every q-chunk.  q/k/v leave this phase x128-scaled.
                bf = (jt == 0)
                if bf:
                    xt = xt0
                else:
                    xt = xtp.tile([128, KC, 512], F8E4, tag="xt", name=f"xt{jt}")
                    xr = xT[:, jt * 512:(jt + 1) * 512].rearrange("(k p) t -> p k t", p=128)
                    for k in range(KC):
                        (nc.sync if k % 2 == 0 else nc.scalar).dma_start(xt[:, k, :], xr[:, k, :])
                wqs, wks, wvs = (
                    (wq16_sb, wk16_sb, wv16_sb) if bf else (wq_sb, wk_sb, wv_sb)
                )

                def mm_acc(p, w_sb, tsl):
                    if bf:
                        for k in range(KC):
                            nc.tensor.matmul(
                                p, w_sb[:, k, tsl], xt[:, k, :],
                                start=(k == 0), stop=(k == KC - 1),
                            )
                    else:
                        for k in range(0, KC, 2):
                            nc.tensor.matmul(
                                p, w_sb[:, k:k + 2, tsl], xt[:, k:k + 2, :],
                                start=(k == 0), stop=(k == KC - 2), perf_mode=DR,
                            )

                for t in range(NP):
                    p = ps.tile([128, 512], F32, tag="st", name=f"pq{jt}_{t}")
                    mm_acc(p, wqs, slice(t * 128, (t + 1) * 128))
                    if bf:
                        nc.vector.tensor_scalar(
                            qT[t][:, jt * 512:(jt + 1) * 512], p,
                            WS, bq_sb[:, t:t + 1], ALU.mult, ALU.add,
                        )
                    else:
                        nc.vector.tensor_scalar_add(
                            qT[t][:, jt * 512:(jt + 1) * 512], p, bq_sb[:, t:t + 1]
                        )
                for t in range(NP):
                    p = ps.tile([128, 512], F32, tag="st", name=f"pk{jt}_{t}")
                    mm_acc(p, wks, slice(t * 128, (t + 1) * 128))
                    if bf:
                        nc.vector.tensor_scalar(
                            kT[t][:, jt * 512:(jt + 1) * 512], p,
                            WS, bk_sb[:, t:t + 1], ALU.mult, ALU.add,
                        )
                    else:
                        nc.vector.tensor_scalar_add(
                            kT[t][:, jt * 512:(jt + 1) * 512], p, bk_sb[:, t:t + 1]
                        )
                for s in range(4):
                    ik = jt * 4 + s
                    p = ps.tile([128, 512], F32, tag="st", name=f"pv{ik}")
                    if bf:
                        for k in range(KC):
                            nc.tensor.matmul(
                                p, xt[:, k, s * 128:(s + 1) * 128], wvs[:, k, :],
                                start=(k == 0), stop=(k == KC - 1),
                            )
                    else:
                        for k in range(0, KC, 2):
                            nc.tensor.matmul(
                                p, xt[:, k:k + 2, s * 128:(s + 1) * 128], wvs[:, k:k + 2, :],
                                start=(k == 0), stop=(k == KC - 2), perf_mode=DR,
                            )
                    vg = v_sb[ik].rearrange("p (h c) -> p h c", c=65)
                    pg = p.rearrange("p (h c) -> p h c", c=64)
                    if bf:
                        nc.vector.tensor_scalar_mul(vg[:, :, 0:64], pg, WS)
                    else:
                        nc.vector.tensor_copy(vg[:, :, 0:64], pg)
                    nc.vector.memset(vg[:, :, 64:65], 1.0)

            def av(t, ik, nik, pts, o_ps):
                pt, c0 = pts[ik]
                ptg = pt.rearrange("p (h q) -> p h q", q=512)
                for hh in range(2):
                    h = 2 * t + hh
                    nc.tensor.matmul(
                        o_ps[hh][:, c0:512], v_sb[ik][:, h * 65:h * 65 + 65],
                        ptg[:, hh, c0:512],
                        start=(ik == 0), stop=(ik == nik - 1),
                    )

            def attention(t, jq):
                nik = 4 * jq + 4
                o_ps = [
                    ps.tile([65, 512], F32, tag="ot", bufs=4, name=f"ops{t}_{jq}_{_h}")
                    for _h in range(2)
                ]
                pts = {}
                for ik in range(nik):
                    d = ik - 4 * jq
                    c0 = 128 * d if d > 0 else 0   # first potentially-valid column
                    st = ps.tile([128, 1024], F32, tag="st", name=f"st{t}_{jq}_{ik}")
                    stg = st.rearrange("p (h q) -> p h q", q=512)
                    for hh in range(2):
                        r = slice(hh * 64, hh * 64 + 64)
                        nc.tensor.matmul(
                            stg[:, hh, c0:512],
                            kT[t][r, ik * 128:(ik + 1) * 128],
                            qT[t][r, jq * 512 + c0:(jq + 1) * 512],
                            start=True, stop=True,
                        )
                    pt = ptp.tile([128, 1024], BT16, tag="pt", name=f"pt{t}_{jq}_{ik}")
                    ptg = pt.rearrange("p (h q) -> p h q", q=512)
                    if d >= 0:
                        nc.scalar.activation(
                            ptg[:, :, c0:512], stg[:, :, c0:512], AF.Exp, scale=SC
                        )
                        # in-place tril mask on just the 128-wide diagonal band
                        for hh in range(2):
                            nc.vector.tensor_mul(
                                ptg[:, hh, c0:c0 + 128],
                                ptg[:, hh, c0:c0 + 128], masks_sb,
                            )
                    else:
                        nc.scalar.activation(pt, st, AF.Exp, scale=SC)
                    pts[ik] = (pt, c0)
                    if ik > 0:
                        av(t, ik - 1, nik, pts, o_ps)
                av(t, nik - 1, nik, pts, o_ps)
                # evict Z row + unnormalized O^T, freeing the PSUM accumulators
                out_h = []
                for hh in range(2):
                    ouz = znp.tile([65, 512], F32, tag="ouz", bufs=6, name=f"oz{t}_{jq}_{hh}")
                    nc.vector.tensor_copy(ouz, o_ps[hh])
                    out_h.append(ouz)
                return out_h

            def phase3_m(m):
                    for n in range(2):
                        p = ps.tile([128, 512], F32, tag="st", name=f"py{m}_{n}")
                        for t in range(NP):
                            nc.tensor.matmul(
                                p, oT[t][:, m * 128:(m + 1) * 128],
                                wo_sb[:, t, n * 512:(n + 1) * 512],
                                start=(t == 0), stop=(t == NP - 1),
                            )
                        ys = ystp.tile([128, 512], F32, tag="y", name=f"ys{m}_{n}")
                        nc.vector.tensor_copy(ys, p)
                        nc.gpsimd.dma_start(
                            y[m * 128:(m + 1) * 128, n * 512:(n + 1) * 512], ys
                        )

            def phase3(jq):
                for m in range(4 * jq, 4 * jq + 4):
                    phase3_m(m)

            import concourse.bass as bass_mod

            def normalize(t, jq, evicted, interleave_phase3=False):
                # evicted: [(ouz_h0, ...), (ouz_h1, ...)] for pair t at chunk jq.
                # Pack both heads' Z rows [1,512] as [8,64] each -> one [16,64]
                # reciprocal (64 elems/lane), then broadcast 1/(128 Z) via a
                # DRAM round-trip (partition-step-0 DMA reads are legal from
                # DRAM).  The x1/128 undoes the v weight pre-scale.
                qs2 = slice(jq * 512, (jq + 1) * 512)
                zb = znp.tile([16, 64], F32, tag="zb", bufs=2, name=f"zb{t}_{jq}")
                for hh in range(2):
                    ouz = evicted[hh]
                    nc.sync.dma_start(
                        zb[8 * hh:8 * hh + 8, :],
                        ouz[64:65, :].rearrange("o (p q) -> o p q", p=8),
                    )
                rcp = znp.tile([16, 64], F32, tag="rcpb", bufs=2, name=f"rcp{t}_{jq}")
                nc.vector.reciprocal(rcp, zb)
                rcp16 = znp.tile([16, 64], BT16, tag="rcp16b", bufs=2, name=f"rcp16{t}_{jq}")
                nc.vector.tensor_scalar_mul(rcp16, rcp, 1.0 / WS)
                for hh in range(2):
                    nc.sync.dma_start(
                        rcp_dram[jq, 2 * t + hh, :].rearrange("(p q) -> p q", p=8),
                        rcp16[8 * hh:8 * hh + 8, :],
                    )
                tmps = []
                for hh in range(2):
                    ouz = evicted[hh]
                    bc_sb = znp.tile([64, 512], BT16, tag="bc_sb", bufs=3, name=f"bs{t}_{jq}_{hh}")
                    src = rcp_dram[jq, 2 * t + hh, :]
                    bcast = bass_mod.AP(
                        tensor=src.tensor, offset=src.offset,
                        ap=[[0, 64]] + [list(a) for a in src.ap],
                    )
                    nc.sync.dma_start(bc_sb, bcast)
                    tmps.append((ouz, bc_sb))
                if not interleave_phase3:
                    for hh, (ouz, bc_sb) in enumerate(tmps):
                        if hh == 0:
                            nc.vector.tensor_mul(oT[t][0:64, qs2], ouz[0:64, :], bc_sb)
                        else:
                            tmp = znp.tile([64, 512], BT16, tag="tmp_o", bufs=2, name=f"tm{t}_{jq}")
                            nc.vector.tensor_mul(tmp, ouz[0:64, :], bc_sb)
                            nc.gpsimd.dma_start(oT[t][64:128, qs2], tmp)
                else:
                    # last pair of the last chunk: per-128-col muls, phase3
                    # m-chunk follows immediately after its slice is ready
                    for mi in range(4):
                        cs = slice(mi * 128, (mi + 1) * 128)
                        gs = slice(jq * 512 + mi * 128, jq * 512 + (mi + 1) * 128)
                        for hh, (ouz, bc_sb) in enumerate(tmps):
                            if hh == 0:
                                nc.vector.tensor_mul(oT[t][0:64, gs], ouz[0:64, cs], bc_sb[:, cs])
                            else:
                                tmp = znp.tile([64, 128], BT16, tag="tmp_os", bufs=4, name=f"tms{t}_{jq}_{mi}")
                                nc.vector.tensor_mul(tmp, ouz[0:64, cs], bc_sb[:, cs])
                                nc.sync.dma_start(oT[t][64:128, gs], tmp)
                        phase3_m(4 * jq + mi)

            phase1(0)
            pend = []          # (t, jq, evicted) not yet normalized
            for jq in range(NJQ):
                for t in range(NP):
                    ev = attention(t, jq)
                    if jq == 0 and t == 0 and NJQ > 1:
                        phase1(1)
                    if pend:
                        pt_, pjq_, pev_ = pend.pop(0)
                        normalize(pt_, pjq_, pev_)
                        if pt_ == NP - 1:
                            phase3(pjq_)
                    pend.append((t, jq, ev))
                if jq + 2 < NJQ:
                    phase1(jq + 2)
            # tail: all but the last pending entry normally; the last one
            # interleaves its normalization with phase3 m-chunks
            for pt_, pjq_, pev_ in pend[:-1]:
                normalize(pt_, pjq_, pev_)
                if pt_ == NP - 1:
                    phase3(pjq_)
            pt_, pjq_, pev_ = pend[-1]
            normalize(pt_, pjq_, pev_, interleave_phase3=True)

    nc.compile()
    return nc


def _host_prep(x, wq, bq, wk, bk, wv, wo):
    qn = np.arange(128)[None, :]
    kn = np.arange(128)[:, None]
    masks_np = (qn >= kn).astype(BF16)

    per_g = []
    for g in range(G):
        cs = slice(g * HD, (g + 1) * HD)
        per_g.append({
            "wq": np.ascontiguousarray(wq[:, cs] * WS).astype(FP8),
            "wk": np.ascontiguousarray(wk[:, cs] * WS).astype(FP8),
            "wv": np.ascontiguousarray(wv[:, cs] * WS).astype(FP8),
            "wq16": np.ascontiguousarray(wq[:, cs]).astype(BF16),
            "wk16": np.ascontiguousarray(wk[:, cs]).astype(BF16),
            "wv16": np.ascontiguousarray(wv[:, cs]).astype(BF16),
            "wo": np.ascontiguousarray(wo[cs, :]).astype(BF16),
            "bq": np.ascontiguousarray((bq[cs] * WS).reshape(NP, 128).T).astype(np.float32),
            "bk": np.ascontiguousarray((bk[cs] * WS).reshape(NP, 128).T).astype(np.float32),
            "masks": masks_np,
        })
    in_maps = []
    for c in range(8):
        b, g = divmod(c, G)
        m = dict(per_g[g])
        xt_full = np.ascontiguousarray(x[b].T)
        m["xT"] = xt_full.astype(FP8)
        m["xT16"] = np.ascontiguousarray(xt_full[:, 0:512]).astype(BF16)
        in_maps.append(m)
    return in_maps


def kernel(x, wq, bq, wk, bk, wv, bv, wo, bo):
    x = np.asarray(x, dtype=np.float32)
    wq = np.asarray(wq, dtype=np.float32)
    bq = np.asarray(bq, dtype=np.float32)
    wk = np.asarray(wk, dtype=np.float32)
    bk = np.asarray(bk, dtype=np.float32)
    wv = np.asarray(wv, dtype=np.float32)
    bv = np.asarray(bv, dtype=np.float32)
    wo = np.asarray(wo, dtype=np.float32)
    bo = np.asarray(bo, dtype=np.float32)

    if "nc" not in _CACHED:
        _CACHED["nc"] = _build()
    nc = _CACHED["nc"]

    in_maps = _host_prep(x, wq, bq, wk, bk, wv, wo)
    res = run_bass_kernel_spmd(nc, in_maps, core_ids=list(range(8)))

    const_row = (bo.astype(np.float64) + bv.astype(np.float64) @ wo.astype(np.float64))
    out = np.empty((B, T, C), dtype=np.float32)
    for b in range(B):
        acc = res.results[2 * b]["y"].astype(np.float64)
        acc += res.results[2 * b + 1]["y"]
        acc += const_row[None, :]
        out[b] = acc.astype(np.float32)
    return out


# revision 14
# speedup vs baseline: 1.3039x; 1.1482x over previous
"""Causal self-attention (B=4, T=2048, C=1024, H=16, D=64) on 8 trn2 NeuronCores.

Sharding: core c = (batch b = c//2, head-group g = c%2). Megatron-style within a
batch: each core computes 8 heads' q/k/v (column-parallel) and a row-parallel
partial out-projection. Host sums the two partials per batch and adds the
rank-1 bias term (bo + bv @ wo) -- valid because softmax rows sum to 1, so v's
bias never needs to enter the kernel.

Per-core kernel:
  phase 1 (per 512-wide T chunk): qT,kT = (x@w)^T.  Chunks 1-3 use fp8e4
           DoubleRow matmuls (weights host-prescaled x128 to clear e4m3
           subnormals; x in fp8e4); chunk 0 runs in bf16 because early
           attention rows (tiny softmax support) get no error averaging and
           kT tiles 0-3 feed every q-chunk.  q/k/v all stay x128-scaled in
           SBUF -- the descale is folded into exp's free scale immediate
           (0.125/128^2) and the 1/Z reciprocal (x 1/128), so evictions are
           plain add/copy ops.  v gets a ones column appended per head.
  phase 2: flash-style streaming attention in S^T orientation:
           S^T[k,q] = kT.T @ qT bf16 (head pairs in PE row groups 0/64 run
           concurrently via row tiling), P^T = exp(S^T * 2^-17) on ScalarE;
           causal masking via an in-place [128,128] tril multiply on only
           the diagonal band; O^T accumulated via lhsT=v_tile (stationary),
           rhs=P^T; the ones column of v makes PSUM row 64 the softmax
           denominator Z for free.  1/(128 Z) broadcast across partitions
           with a DRAM round-trip (partition-step-0 DMA reads are legal
           from DRAM).
  phase 3 (per T chunk, overlapped with the next chunk's attention):
           y = O @ wo bf16 via lhsT=O^T (already the natural layout).
"""
import numpy as np
import ml_dtypes

import concourse.tile as tile
from concourse import bacc, mybir
from concourse.bass_utils import run_bass_kernel_spmd

BF16 = ml_dtypes.bfloat16
FP8 = ml_dtypes.float8_e4m3
F32 = mybir.dt.float32
BT16 = mybir.dt.bfloat16
F8E4 = mybir.dt.float8e4
AF = mybir.ActivationFunctionType
ALU = mybir.AluOpType
DR = mybir.MatmulPerfMode.DoubleRow

B, T, C, H, D = 4, 2048, 1024, 16, 64
G = 2              # head groups (cores per batch)
HL = H // G        # heads per core = 8
HD = HL * D        # local head dims = 512
NP = 4             # head pairs per core
NJQ = T // 512     # q chunks of 512 = 4
NIK = T // 128     # k tiles of 128 = 16
KC = C // 128      # contraction chunks = 8
WS = 128.0         # host pre-scale on fp8 weights
SC = 0.125 / (WS * WS)   # exp scale: 1/sqrt(D) and the two x128 descales

_CACHED = {}


def _build():
    nc = bacc.Bacc("TRN2", debug=False)
    xT = nc.dram_tensor("xT", [C, T], F8E4, kind="ExternalInput").ap()
    xT16 = nc.dram_tensor("xT16", [C, 512], BT16, kind="ExternalInput").ap()
    wq = nc.dram_tensor("wq", [C, HD], F8E4, kind="ExternalInput").ap()
    wk = nc.dram_tensor("wk", [C, HD], F8E4, kind="ExternalInput").ap()
    wv = nc.dram_tensor("wv", [C, HD], F8E4, kind="ExternalInput").ap()
    wq16 = nc.dram_tensor("wq16", [C, HD], BT16, kind="ExternalInput").ap()
    wk16 = nc.dram_tensor("wk16", [C, HD], BT16, kind="ExternalInput").ap()
    wv16 = nc.dram_tensor("wv16", [C, HD], BT16, kind="ExternalInput").ap()
    wo = nc.dram_tensor("wo", [HD, C], BT16, kind="ExternalInput").ap()
    bq = nc.dram_tensor("bq", [128, NP], F32, kind="ExternalInput").ap()
    bk = nc.dram_tensor("bk", [128, NP], F32, kind="ExternalInput").ap()
    masks = nc.dram_tensor("masks", [128, 128], BT16, kind="ExternalInput").ap()
    rcp_dram = nc.dram_tensor("rcp_dram", [NJQ, 8, 512], BT16).ap()
    y = nc.dram_tensor("y", [T, C], F32, kind="ExternalOutput").ap()

    with tile.TileContext(nc) as tc:
        with (
            tc.tile_pool(name="consts", bufs=1) as consts,
            tc.tile_pool(name="xt", bufs=3) as xtp,
            tc.tile_pool(name="qk", bufs=1) as qkp,
            tc.tile_pool(name="vp", bufs=1) as vp,
            tc.tile_pool(name="otp", bufs=1) as otp,
            tc.tile_pool(name="pt", bufs=6) as ptp,
            tc.tile_pool(name="zn", bufs=3) as znp,
            tc.tile_pool(name="yst", bufs=4) as ystp,
            tc.tile_pool(name="ps", bufs=2, space="PSUM") as ps,
        ):
            # ---- constants (biases are tiny and gate evictions: load them first) ----
            bq_dma = consts.tile([128, NP], F32, tag="bq_dma")
            bq_sb = consts.tile([128, NP], F32, tag="bq")
            nc.sync.dma_start(bq_dma, bq)
            nc.vector.tensor_copy(bq_sb, bq_dma)
            bk_dma = consts.tile([128, NP], F32, tag="bk_dma")
            bk_sb = consts.tile([128, NP], F32, tag="bk")
            nc.sync.dma_start(bk_dma, bk)
            nc.vector.tensor_copy(bk_sb, bk_dma)
            # bf16 weights + chunk-0 x first: phase1(0) runs in bf16
            wq16_sb = consts.tile([128, KC, HD], BT16, tag="wq16")
            wq16_r = wq16.rearrange("(k p) c -> p k c", p=128)
            xt0 = xtp.tile([128, KC, 512], BT16, tag="xt16", name="xt_pre0")
            x0_r = xT16.rearrange("(k p) t -> p k t", p=128)
            for k in range(KC):
                nc.scalar.dma_start(wq16_sb[:, k, :], wq16_r[:, k, :])
                nc.sync.dma_start(xt0[:, k, :], x0_r[:, k, :])
            wk16_sb = consts.tile([128, KC, HD], BT16, tag="wk16")
            wk16_r = wk16.rearrange("(k p) c -> p k c", p=128)
            for k in range(KC):
                nc.sync.dma_start(wk16_sb[:, k, :], wk16_r[:, k, :])
            masks_dma = consts.tile([128, 128], BT16, tag="masks_dma")
            masks_sb = consts.tile([128, 128], BT16, tag="masks")
            nc.gpsimd.dma_start(masks_dma, masks)
            nc.gpsimd.tensor_copy(masks_sb, masks_dma)
            wv16_sb = consts.tile([128, KC, HD], BT16, tag="wv16")
            nc.gpsimd.dma_start(wv16_sb, wv16.rearrange("(k p) c -> p k c", p=128))
            wq_sb = consts.tile([128, KC, HD], F8E4, tag="wq")
            nc.scalar.dma_start(wq_sb, wq.rearrange("(k p) c -> p k c", p=128))
            wk_sb = consts.tile([128, KC, HD], F8E4, tag="wk")
            nc.sync.dma_start(wk_sb, wk.rearrange("(k p) c -> p k c", p=128))
            wv_sb = consts.tile([128, KC, HD], F8E4, tag="wv")
            nc.gpsimd.dma_start(wv_sb, wv.rearrange("(k p) c -> p k c", p=128))
            wo_sb = consts.tile([128, NP, C], BT16, tag="wo")
            nc.gpsimd.dma_start(wo_sb, wo.rearrange("(t p) c -> p t c", p=128))
            # ---- persistent activations ----
            qT = [qkp.tile([128, T], BT16, tag=f"qT{t}", name=f"qT{t}") for t in range(NP)]
            kT = [qkp.tile([128, T], BT16, tag=f"kT{t}", name=f"kT{t}") for t in range(NP)]
            v_sb = [vp.tile([128, HL * 65], BT16, tag=f"v{i}", name=f"v{i}") for i in range(NIK)]
            oT = [otp.tile([128, T], BT16, tag=f"oT{t}", name=f"oT{t}") for t in range(NP)]

            def phase1(jt):
                # jt==0 runs in bf16: early attention rows (small softmax
                # support) get no error averaging, and kT tiles 0-3 feed
                # every q-chunk.  q/k/v leave this phase x128-scaled.
                bf = (jt == 0)
                if bf:
                    xt = xt0
                else:
                    xt = xtp.tile([128, KC, 512], F8E4, tag="xt", name=f"xt{jt}")
                    xr = xT[:, jt * 512:(jt + 1) * 512].rearrange("(k p) t -> p k t", p=128)
                    for k in range(KC):
                        (nc.sync if k % 2 == 0 else nc.scalar).dma_start(xt[:, k, :], xr[:, k, :])
                wqs, wks, wvs = (
                    (wq16_sb, wk16_sb, wv16_sb) if bf else (wq_sb, wk_sb, wv_sb)
                )

                def mm_acc(p, w_sb, tsl):
                    if bf:
                        for k in range(KC):
                            nc.tensor.matmul(
                                p, w_sb[:, k, tsl], xt[:, k, :],
                                start=(k == 0), stop=(k == KC - 1),
                            )
                    else:
                        for k in range(0, KC, 2):
                            nc.tensor.matmul(
                                p, w_sb[:, k:k + 2, tsl], xt[:, k:k + 2, :],
                                start=(k == 0), stop=(k == KC - 2), perf_mode=DR,
                            )

                for t in range(NP):
                    p = ps.tile([128, 512], F32, tag="st", name=f"pq{jt}_{t}")
                    mm_acc(p, wqs, slice(t * 128, (t + 1) * 128))
                    nc.vector.tensor_scalar(
                        qT[t][:, jt * 512:(jt + 1) * 512], p,
                        0.125 if bf else 0.125 / WS, bq_sb[:, t:t + 1],
                        ALU.mult, ALU.add,
                    )
                for t in range(NP):
                    p = ps.tile([128, 512], F32, tag="st", name=f"pk{jt}_{t}")
                    mm_acc(p, wks, slice(t * 128, (t + 1) * 128))
                    if bf:
                        nc.vector.tensor_scalar_add(
                            kT[t][:, jt * 512:(jt + 1) * 512], p, bk_sb[:, t:t + 1]
                        )
                    else:
                        nc.vector.tensor_scalar(
                            kT[t][:, jt * 512:(jt + 1) * 512], p,
                            1.0 / WS, bk_sb[:, t:t + 1], ALU.mult, ALU.add,
                        )
                for s in range(4):
                    ik = jt * 4 + s
                    p = ps.tile([128, 512], F32, tag="st", name=f"pv{ik}")
                    if bf:
                        for k in range(KC):
                            nc.tensor.matmul(
                                p, xt[:, k, s * 128:(s + 1) * 128], wvs[:, k, :],
                                start=(k == 0), stop=(k == KC - 1),
                            )
                    else:
                        for k in range(0, KC, 2):
                            nc.tensor.matmul(
                                p, xt[:, k:k + 2, s * 128:(s + 1) * 128], wvs[:, k:k + 2, :],
                                start=(k == 0), stop=(k == KC - 2), perf_mode=DR,
                            )
                    vg = v_sb[ik].rearrange("p (h c) -> p h c", c=65)
                    pg = p.rearrange("p (h c) -> p h c", c=64)
                    if bf:
                        nc.vector.tensor_scalar_mul(vg[:, :, 0:64], pg, WS)
                    else:
                        nc.vector.tensor_copy(vg[:, :, 0:64], pg)
                    nc.vector.memset(vg[:, :, 64:65], 1.0)

            def av(t, ik, nik, pts, o_ps):
                pt, c0 = pts[ik]
                ptg = pt.rearrange("p (h q) -> p h q", q=512)
                for hh in range(2):
                    h = 2 * t + hh
                    nc.tensor.matmul(
                        o_ps[hh][:, c0:512], v_sb[ik][:, h * 65:h * 65 + 65],
                        ptg[:, hh, c0:512],
                        start=(ik == 0), stop=(ik == nik - 1),
                    )

            def attention(t, jq):
                nik = 4 * jq + 4
                o_ps = [
                    ps.tile([65, 512], F32, tag="ot", bufs=4, name=f"ops{t}_{jq}_{_h}")
                    for _h in range(2)
                ]
                pts = {}
                for ik in range(nik):
                    d = ik - 4 * jq
                    c0 = 128 * d if d > 0 else 0   # first potentially-valid column
                    st = ps.tile([128, 1024], F32, tag="st", name=f"st{t}_{jq}_{ik}")
                    stg = st.rearrange("p (h q) -> p h q", q=512)
                    for hh in range(2):
                        r = slice(hh * 64, hh * 64 + 64)
                        nc.tensor.matmul(
                            stg[:, hh, c0:512],
                            kT[t][r, ik * 128:(ik + 1) * 128],
                            qT[t][r, jq * 512 + c0:(jq + 1) * 512],
                            start=True, stop=True,
                        )
                    pt = ptp.tile([128, 1024], BT16, tag="pt", name=f"pt{t}_{jq}_{ik}")
                    ptg = pt.rearrange("p (h q) -> p h q", q=512)
                    if d >= 0:
                        nc.scalar.activation(ptg[:, :, c0:512], stg[:, :, c0:512], AF.Exp)
                        # in-place tril mask on just the 128-wide diagonal band
                        for hh in range(2):
                            nc.vector.tensor_mul(
                                ptg[:, hh, c0:c0 + 128],
                                ptg[:, hh, c0:c0 + 128], masks_sb,
                            )
                    else:
                        nc.scalar.activation(pt, st, AF.Exp)
                    pts[ik] = (pt, c0)
                    if ik > 0:
                        av(t, ik - 1, nik, pts, o_ps)
                av(t, nik - 1, nik, pts, o_ps)
                # evict Z row + unnormalized O^T, freeing the PSUM accumulators
                out_h = []
                for hh in range(2):
                    ouz = znp.tile([65, 512], F32, tag="ouz", bufs=6, name=f"oz{t}_{jq}_{hh}")
                    nc.vector.tensor_copy(ouz, o_ps[hh])
                    out_h.append(ouz)
                return out_h

            def phase3_m(m):
                    for n in range(2):
                        p = ps.tile([128, 512], F32, tag="st", name=f"py{m}_{n}")
                        for t in range(NP):
                            nc.tensor.matmul(
                                p, oT[t][:, m * 128:(m + 1) * 128],
                                wo_sb[:, t, n * 512:(n + 1) * 512],
                                start=(t == 0), stop=(t == NP - 1),
                            )
                        ys = ystp.tile([128, 512], F32, tag="y", name=f"ys{m}_{n}")
                        nc.vector.tensor_copy(ys, p)
                        nc.gpsimd.dma_start(
                            y[m * 128:(m + 1) * 128, n * 512:(n + 1) * 512], ys
                        )

            def phase3(jq):
                for m in range(4 * jq, 4 * jq + 4):
                    phase3_m(m)

            import concourse.bass as bass_mod

            def normalize(t, jq, evicted, interleave_phase3=False):
                # evicted: [(ouz_h0, ...), (ouz_h1, ...)] for pair t at chunk jq.
                # Pack both heads' Z rows [1,512] as [8,64] each -> one [16,64]
                # reciprocal (64 elems/lane), then broadcast 1/(128 Z) via a
                # DRAM round-trip (partition-step-0 DMA reads are legal from
                # DRAM).  The x1/128 undoes the v weight pre-scale.
                qs2 = slice(jq * 512, (jq + 1) * 512)
                zb = znp.tile([16, 64], F32, tag="zb", bufs=2, name=f"zb{t}_{jq}")
                for hh in range(2):
                    ouz = evicted[hh]
                    nc.sync.dma_start(
                        zb[8 * hh:8 * hh + 8, :],
                        ouz[64:65, :].rearrange("o (p q) -> o p q", p=8),
                    )
                rcp = znp.tile([16, 64], F32, tag="rcpb", bufs=2, name=f"rcp{t}_{jq}")
                nc.vector.reciprocal(rcp, zb)
                rcp16 = znp.tile([16, 64], BT16, tag="rcp16b", bufs=2, name=f"rcp16{t}_{jq}")
                nc.vector.tensor_scalar_mul(rcp16, rcp, 1.0 / WS)
                for hh in range(2):
                    nc.sync.dma_start(
                        rcp_dram[jq, 2 * t + hh, :].rearrange("(p q) -> p q", p=8),
                        rcp16[8 * hh:8 * hh + 8, :],
                    )
                tmps = []
                for hh in range(2):
                    ouz = evicted[hh]
                    bc_sb = znp.tile([64, 512], BT16, tag="bc_sb", bufs=3, name=f"bs{t}_{jq}_{hh}")
                    src = rcp_dram[jq, 2 * t + hh, :]
                    bcast = bass_mod.AP(
                        tensor=src.tensor, offset=src.offset,
                        ap=[[0, 64]] + [list(a) for a in src.ap],
                    )
                    nc.sync.dma_start(bc_sb, bcast)
                    tmps.append((ouz, bc_sb))
                if not interleave_phase3:
                    for hh, (ouz, bc_sb) in enumerate(tmps):
                        if hh == 0:
                            nc.vector.tensor_mul(oT[t][0:64, qs2], ouz[0:64, :], bc_sb)
                        else:
                            tmp = znp.tile([64, 512], BT16, tag="tmp_o", bufs=2, name=f"tm{t}_{jq}")
                            nc.vector.tensor_mul(tmp, ouz[0:64, :], bc_sb)
                            nc.gpsimd.dma_start(oT[t][64:128, qs2], tmp)
                else:
                    # last pair of the last chunk: per-128-col muls, phase3
                    # m-chunk follows immediately after its slice is ready
                    for mi in range(4):
                        cs = slice(mi * 128, (mi + 1) * 128)
                        gs = slice(jq * 512 + mi * 128, jq * 512 + (mi + 1) * 128)
                        for hh, (ouz, bc_sb) in enumerate(tmps):
                            if hh == 0:
                                nc.vector.tensor_mul(oT[t][0:64, gs], ouz[0:64, cs], bc_sb[:, cs])
                            else:
                                tmp = znp.tile([64, 128], BT16, tag="tmp_os", bufs=4, name=f"tms{t}_{jq}_{mi}")
                                nc.vector.tensor_mul(tmp, ouz[0:64, cs], bc_sb[:, cs])
                                nc.sync.dma_start(oT[t][64:128, gs], tmp)
                        phase3_m(4 * jq + mi)

            phase1(0)
            pend = []          # (t, jq, evicted) not yet normalized
            for jq in range(NJQ):
                for t in range(NP):
                    ev = attention(t, jq)
                    if jq == 0 and t == 0 and NJQ > 1:
                        phase1(1)
                    if pend:
                        pt_, pjq_, pev_ = pend.pop(0)
                        normalize(pt_, pjq_, pev_)
                        if pt_ == NP - 1:
                            phase3(pjq_)
                    pend.append((t, jq, ev))
                if jq + 2 < NJQ:
                    phase1(jq + 2)
            # tail: all but the last pending entry normally; the last one
            # interleaves its normalization with phase3 m-chunks
            for pt_, pjq_, pev_ in pend[:-1]:
                normalize(pt_, pjq_, pev_)
                if pt_ == NP - 1:
                    phase3(pjq_)
            pt_, pjq_, pev_ = pend[-1]
            normalize(pt_, pjq_, pev_, interleave_phase3=True)

    nc.compile()
    return nc


def _host_prep(x, wq, bq, wk, bk, wv, wo):
    qn = np.arange(128)[None, :]
    kn = np.arange(128)[:, None]
    masks_np = (qn >= kn).astype(BF16)

    per_g = []
    for g in range(G):
        cs = slice(g * HD, (g + 1) * HD)
        per_g.append({
            "wq": np.ascontiguousarray(wq[:, cs] * WS).astype(FP8),
            "wk": np.ascontiguousarray(wk[:, cs] * WS).astype(FP8),
            "wv": np.ascontiguousarray(wv[:, cs] * WS).astype(FP8),
            "wq16": np.ascontiguousarray(wq[:, cs]).astype(BF16),
            "wk16": np.ascontiguousarray(wk[:, cs]).astype(BF16),
            "wv16": np.ascontiguousarray(wv[:, cs]).astype(BF16),
            "wo": np.ascontiguousarray(wo[cs, :]).astype(BF16),
            "bq": np.ascontiguousarray((bq[cs] / 8.0).reshape(NP, 128).T).astype(np.float32),
            "bk": np.ascontiguousarray(bk[cs].reshape(NP, 128).T).astype(np.float32),
            "masks": masks_np,
        })
    in_maps = []
    for c in range(8):
        b, g = divmod(c, G)
        m = dict(per_g[g])
        xt_full = np.ascontiguousarray(x[b].T)
        m["xT"] = xt_full.astype(FP8)
        m["xT16"] = np.ascontiguousarray(xt_full[:, 0:512]).astype(BF16)
        in_maps.append(m)
    return in_maps


def kernel(x, wq, bq, wk, bk, wv, bv, wo, bo):
    x = np.asarray(x, dtype=np.float32)
    wq = np.asarray(wq, dtype=np.float32)
    bq = np.asarray(bq, dtype=np.float32)
    wk = np.asarray(wk, dtype=np.float32)
    bk = np.asarray(bk, dtype=np.float32)
    wv = np.asarray(wv, dtype=np.float32)
    bv = np.asarray(bv, dtype=np.float32)
    wo = np.asarray(wo, dtype=np.float32)
    bo = np.asarray(bo, dtype=np.float32)

    if "nc" not in _CACHED:
        _CACHED["nc"] = _build()
    nc = _CACHED["nc"]

    in_maps = _host_prep(x, wq, bq, wk, bk, wv, wo)
    res = run_bass_kernel_spmd(nc, in_maps, core_ids=list(range(8)))

    const_row = (bo.astype(np.float64) + bv.astype(np.float64) @ wo.astype(np.float64))
    out = np.empty((B, T, C), dtype=np.float32)
    for b in range(B):
        acc = res.results[2 * b]["y"].astype(np.float64)
        acc += res.results[2 * b + 1]["y"]
        acc += const_row[None, :]
        out[b] = acc.astype(np.float32)
    return out


# revision 20
# speedup vs baseline: 1.4037x; 1.0766x over previous
"""Causal self-attention (B=4, T=2048, C=1024, H=16, D=64) on 8 trn2 NeuronCores.

Sharding: core c = (batch b = c//2, head-group g = c%2). Megatron-style within a
batch: each core computes 8 heads' q/k/v (column-parallel) and a row-parallel
partial out-projection. Host sums the two partials per batch and adds the
rank-1 bias term (bo + bv @ wo) -- valid because softmax rows sum to 1, so v's
bias never needs to enter the kernel.

Per-core kernel:
  phase 1 (per 512-wide T chunk): qT,kT = (x@w)^T.  Chunks 1-3 use fp8e4
           DoubleRow matmuls (weights host-prescaled x128 to clear e4m3
           subnormals; x in fp8e4); chunk 0 runs in bf16 because early
           attention rows (tiny softmax support) get no error averaging and
           kT tiles 0-3 feed every q-chunk.  q/k/v all stay x128-scaled in
           SBUF -- the descale is folded into exp's free scale immediate
           (0.125/128^2) and the 1/Z reciprocal (x 1/128), so evictions are
           plain add/copy ops.  v gets a ones column appended per head.
  phase 2: flash-style streaming attention in S^T orientation:
           S^T[k,q] = kT.T @ qT bf16 (head pairs in PE row groups 0/64 run
           concurrently via row tiling), P^T = exp(S^T * 2^-17) on ScalarE;
           causal masking via an in-place [128,128] tril multiply on only
           the diagonal band; O^T accumulated via lhsT=v_tile (stationary),
           rhs=P^T; the ones column of v makes PSUM row 64 the softmax
           denominator Z for free.  1/(128 Z) broadcast across partitions
           with a DRAM round-trip (partition-step-0 DMA reads are legal
           from DRAM).
  phase 3 (per T chunk, overlapped with the next chunk's attention):
           y = O @ wo bf16 via lhsT=O^T (already the natural layout).
"""
import numpy as np
import ml_dtypes

import concourse.tile as tile
from concourse import bacc, mybir
from concourse.bass_utils import run_bass_kernel_spmd

BF16 = ml_dtypes.bfloat16
FP8 = ml_dtypes.float8_e4m3
F32 = mybir.dt.float32
BT16 = mybir.dt.bfloat16
F8E4 = mybir.dt.float8e4
AF = mybir.ActivationFunctionType
ALU = mybir.AluOpType
DR = mybir.MatmulPerfMode.DoubleRow

B, T, C, H, D = 4, 2048, 1024, 16, 64
G = 2              # head groups (cores per batch)
HL = H // G        # heads per core = 8
HD = HL * D        # local head dims = 512
NP = 4             # head pairs per core
NJQ = T // 512     # q chunks of 512 = 4
NIK = T // 128     # k tiles of 128 = 16
KC = C // 128      # contraction chunks = 8
WS = 128.0         # host pre-scale on fp8 weights
SC = 0.125 / (WS * WS)   # exp scale: 1/sqrt(D) and the two x128 descales

_CACHED = {}


def _build():
    nc = bacc.Bacc("TRN2", debug=False)
    xT = nc.dram_tensor("xT", [C, T], F8E4, kind="ExternalInput").ap()
    xT16 = nc.dram_tensor("xT16", [C, 512], BT16, kind="ExternalInput").ap()
    wq = nc.dram_tensor("wq", [C, HD], F8E4, kind="ExternalInput").ap()
    wk = nc.dram_tensor("wk", [C, HD], F8E4, kind="ExternalInput").ap()
    wv = nc.dram_tensor("wv", [C, HD], F8E4, kind="ExternalInput").ap()
    wq16 = nc.dram_tensor("wq16", [C, HD], BT16, kind="ExternalInput").ap()
    wk16 = nc.dram_tensor("wk16", [C, HD], BT16, kind="ExternalInput").ap()
    wv16 = nc.dram_tensor("wv16", [C, HD], BT16, kind="ExternalInput").ap()
    wo = nc.dram_tensor("wo", [HD, C], BT16, kind="ExternalInput").ap()
    bq = nc.dram_tensor("bq", [128, NP], F32, kind="ExternalInput").ap()
    bk = nc.dram_tensor("bk", [128, NP], F32, kind="ExternalInput").ap()
    masks = nc.dram_tensor("masks", [128, 128], BT16, kind="ExternalInput").ap()
    rcp_dram = nc.dram_tensor("rcp_dram", [NJQ, 8, 512], BT16).ap()
    y = nc.dram_tensor("y", [T, C], F32, kind="ExternalOutput").ap()

    with tile.TileContext(nc) as tc:
        with (
            tc.tile_pool(name="consts", bufs=1) as consts,
            tc.tile_pool(name="xt", bufs=3) as xtp,
            tc.tile_pool(name="qk", bufs=1) as qkp,
            tc.tile_pool(name="vp", bufs=1) as vp,
            tc.tile_pool(name="otp", bufs=1) as otp,
            tc.tile_pool(name="pt", bufs=6) as ptp,
            tc.tile_pool(name="zn", bufs=3) as znp,
            tc.tile_pool(name="yst", bufs=4) as ystp,
            tc.tile_pool(name="ps", bufs=2, space="PSUM") as ps,
        ):
            # ---- constants (biases are tiny and gate evictions: load them first) ----
            bq_dma = consts.tile([128, NP], F32, tag="bq_dma")
            bq_sb = consts.tile([128, NP], F32, tag="bq")
            nc.sync.dma_start(bq_dma, bq)
            nc.vector.tensor_copy(bq_sb, bq_dma)
            bk_dma = consts.tile([128, NP], F32, tag="bk_dma")
            bk_sb = consts.tile([128, NP], F32, tag="bk")
            nc.sync.dma_start(bk_dma, bk)
            nc.vector.tensor_copy(bk_sb, bk_dma)
            # bf16 weights + chunk-0 x first: phase1(0) runs in bf16
            wq16_sb = consts.tile([128, KC, HD], BT16, tag="wq16")
            wq16_r = wq16.rearrange("(k p) c -> p k c", p=128)
            xt0 = xtp.tile([128, KC, 512], BT16, tag="xt16", name="xt_pre0")
            x0_r = xT16.rearrange("(k p) t -> p k t", p=128)
            for k in range(KC):
                nc.scalar.dma_start(wq16_sb[:, k, :], wq16_r[:, k, :])
                nc.sync.dma_start(xt0[:, k, :], x0_r[:, k, :])
            wk16_sb = consts.tile([128, KC, HD], BT16, tag="wk16")
            wk16_r = wk16.rearrange("(k p) c -> p k c", p=128)
            for k in range(KC):
                nc.sync.dma_start(wk16_sb[:, k, :], wk16_r[:, k, :])
            masks_dma = consts.tile([128, 128], BT16, tag="masks_dma")
            masks_sb = consts.tile([128, 128], BT16, tag="masks")
            nc.gpsimd.dma_start(masks_dma, masks)
            nc.gpsimd.tensor_copy(masks_sb, masks_dma)
            wv16_sb = consts.tile([128, KC, HD], BT16, tag="wv16")
            nc.gpsimd.dma_start(wv16_sb, wv16.rearrange("(k p) c -> p k c", p=128))
            wq_sb = consts.tile([128, KC, HD], F8E4, tag="wq")
            nc.scalar.dma_start(wq_sb, wq.rearrange("(k p) c -> p k c", p=128))
            wk_sb = consts.tile([128, KC, HD], F8E4, tag="wk")
            nc.sync.dma_start(wk_sb, wk.rearrange("(k p) c -> p k c", p=128))
            wv_sb = consts.tile([128, KC, HD], F8E4, tag="wv")
            nc.gpsimd.dma_start(wv_sb, wv.rearrange("(k p) c -> p k c", p=128))
            wo_sb = consts.tile([128, NP, C], BT16, tag="wo")
            nc.gpsimd.dma_start(wo_sb, wo.rearrange("(t p) c -> p t c", p=128))
            bconst = consts.tile([1, 64], BT16, tag="bconst")
            nc.vector.memset(bconst, WS)
            # ---- persistent activations ----
            qT = [qkp.tile([128, T], BT16, tag=f"qT{t}", name=f"qT{t}") for t in range(NP)]
            kT = [qkp.tile([128, T], BT16, tag=f"kT{t}", name=f"kT{t}") for t in range(NP)]
            v_sb = [vp.tile([128, HL * 65], BT16, tag=f"v{i}", name=f"v{i}") for i in range(NIK)]
            oT = [otp.tile([128, T], BT16, tag=f"oT{t}", name=f"oT{t}") for t in range(NP)]

            from collections import deque
            fillers = deque()   # (jt_deadline, closure) granules drained
                                # one-per-attention-tile so phase1/phase3
                                # matmuls interleave finely with the
                                # exp-paced attention stream

            def drain(n=1):
                for _ in range(min(n, len(fillers))):
                    fillers.popleft()[1]()

            def drain_for(jt):
                while any(d is not None and d <= jt for d, _ in fillers):
                    fillers.popleft()[1]()

            def phase1(jt, defer=False):
                # jt==0 runs in bf16: early attention rows (small softmax
                # support) get no error averaging, and kT tiles 0-3 feed
                # every q-chunk.  q/k/v leave this phase x128-scaled.
                bf = (jt == 0)
                if bf:
                    xt = xt0
                else:
                    xt = xtp.tile([128, KC, 512], F8E4, tag="xt", name=f"xt{jt}")
                    xr = xT[:, jt * 512:(jt + 1) * 512].rearrange("(k p) t -> p k t", p=128)
                    for k in range(KC):
                        (nc.sync if k % 2 == 0 else nc.scalar).dma_start(xt[:, k, :], xr[:, k, :])
                wqs, wks, wvs = (
                    (wq16_sb, wk16_sb, wv16_sb) if bf else (wq_sb, wk_sb, wv_sb)
                )

                def mm_acc(p, w_sb, tsl):
                    if bf:
                        for k in range(KC):
                            nc.tensor.matmul(
                                p, w_sb[:, k, tsl], xt[:, k, :],
                                start=(k == 0), stop=(k == KC - 1),
                            )
                    else:
                        for k in range(0, KC, 2):
                            nc.tensor.matmul(
                                p, w_sb[:, k:k + 2, tsl], xt[:, k:k + 2, :],
                                start=(k == 0), stop=(k == KC - 2), perf_mode=DR,
                            )

                ptag = "st" if bf else "ot"

                def q_gran(t):
                    p = ps.tile([128, 512], F32, tag=ptag, bufs=(2 if bf else 4), name=f"pq{jt}_{t}")
                    mm_acc(p, wqs, slice(t * 128, (t + 1) * 128))
                    nc.vector.tensor_scalar(
                        qT[t][:, jt * 512:(jt + 1) * 512], p,
                        0.125 if bf else 0.125 / WS, bq_sb[:, t:t + 1],
                        ALU.mult, ALU.add,
                    )

                def k_gran(t):
                    p = ps.tile([128, 512], F32, tag=ptag, bufs=(2 if bf else 4), name=f"pk{jt}_{t}")
                    mm_acc(p, wks, slice(t * 128, (t + 1) * 128))
                    if bf:
                        nc.vector.tensor_scalar_add(
                            kT[t][:, jt * 512:(jt + 1) * 512], p, bk_sb[:, t:t + 1]
                        )
                    else:
                        nc.vector.tensor_scalar(
                            kT[t][:, jt * 512:(jt + 1) * 512], p,
                            1.0 / WS, bk_sb[:, t:t + 1], ALU.mult, ALU.add,
                        )

                def v_gran(s):
                    ik = jt * 4 + s
                    p = ps.tile([128, 512], F32, tag=ptag, bufs=(2 if bf else 4), name=f"pv{ik}")
                    if bf:
                        for k in range(KC):
                            nc.tensor.matmul(
                                p, xt[:, k, s * 128:(s + 1) * 128], wvs[:, k, :],
                                start=(k == 0), stop=(k == KC - 1),
                            )
                    else:
                        for k in range(0, KC, 2):
                            nc.tensor.matmul(
                                p, xt[:, k:k + 2, s * 128:(s + 1) * 128], wvs[:, k:k + 2, :],
                                start=(k == 0), stop=(k == KC - 2), perf_mode=DR,
                            )
                    vg = v_sb[ik].rearrange("p (h c) -> p h c", c=65)
                    pg = p.rearrange("p (h c) -> p h c", c=64)
                    if bf:
                        nc.vector.tensor_scalar_mul(vg[:, :, 0:64], pg, WS)
                    else:
                        nc.vector.tensor_copy(vg[:, :, 0:64], pg)
                    nc.vector.memset(vg[:, :, 64:65], 1.0)

                grans = (
                    [(lambda t=t: q_gran(t)) for t in range(NP)]
                    + [(lambda t=t: k_gran(t)) for t in range(NP)]
                    + [(lambda s=s: v_gran(s)) for s in range(4)]
                )
                if defer:
                    for g in grans:
                        fillers.append((jt, g))
                else:
                    for g in grans:
                        g()

            def av(t, ik, nik, pts, o_ps):
                pt, c0 = pts[ik]
                ptg = pt.rearrange("p (h q) -> p h q", q=512)
                for hh in range(2):
                    h = 2 * t + hh
                    nc.tensor.matmul(
                        o_ps[hh][:, c0:512], v_sb[ik][:, h * 65:h * 65 + 65],
                        ptg[:, hh, c0:512],
                        start=(ik == 0), stop=(ik == nik - 1),
                    )

            def attention(t, jq):
                nik = 4 * jq + 4
                o_ps = [
                    ps.tile([65, 512], F32, tag="ot", bufs=4, name=f"ops{t}_{jq}_{_h}")
                    for _h in range(2)
                ]
                pts = {}
                for ik in range(nik):
                    d = ik - 4 * jq
                    c0 = 128 * d if d > 0 else 0   # first potentially-valid column
                    st = ps.tile([128, 1024], F32, tag="st", name=f"st{t}_{jq}_{ik}")
                    stg = st.rearrange("p (h q) -> p h q", q=512)
                    for hh in range(2):
                        r = slice(hh * 64, hh * 64 + 64)
                        nc.tensor.matmul(
                            stg[:, hh, c0:512],
                            kT[t][r, ik * 128:(ik + 1) * 128],
                            qT[t][r, jq * 512 + c0:(jq + 1) * 512],
                            start=True, stop=True,
                        )
                    pt = ptp.tile([128, 1024], BT16, tag="pt", name=f"pt{t}_{jq}_{ik}")
                    ptg = pt.rearrange("p (h q) -> p h q", q=512)
                    if d >= 0:
                        nc.scalar.activation(ptg[:, :, c0:512], stg[:, :, c0:512], AF.Exp)
                        # in-place tril mask on just the 128-wide diagonal band
                        for hh in range(2):
                            nc.vector.tensor_mul(
                                ptg[:, hh, c0:c0 + 128],
                                ptg[:, hh, c0:c0 + 128], masks_sb,
                            )
                    else:
                        nc.scalar.activation(pt, st, AF.Exp)
                    pts[ik] = (pt, c0)
                    if ik > 0:
                        av(t, ik - 1, nik, pts, o_ps)
                    drain(1)
                av(t, nik - 1, nik, pts, o_ps)
                # evict Z row + unnormalized O^T, freeing the PSUM accumulators
                out_h = []
                for hh in range(2):
                    ouz = znp.tile([65, 512], F32, tag="ouz", bufs=6, name=f"oz{t}_{jq}_{hh}")
                    nc.vector.tensor_copy(ouz, o_ps[hh])
                    out_h.append(ouz)
                return out_h

            def phase3_mn(m, n, tag="ot"):
                p = ps.tile([128, 512], F32, tag=tag, bufs=(2 if tag == "st" else 4), name=f"py{m}_{n}")
                for t in range(NP):
                    nc.tensor.matmul(
                        p, oT[t][:, m * 128:(m + 1) * 128],
                        wo_sb[:, t, n * 512:(n + 1) * 512],
                        start=(t == 0), stop=(t == NP - 1),
                    )
                ys = ystp.tile([128, 512], F32, tag="y", name=f"ys{m}_{n}")
                nc.vector.tensor_copy(ys, p)
                nc.gpsimd.dma_start(
                    y[m * 128:(m + 1) * 128, n * 512:(n + 1) * 512], ys
                )

            def phase3_m(m):
                for n in range(2):
                    phase3_mn(m, n, tag="st")

            def phase3(jq):
                for m in range(4 * jq, 4 * jq + 4):
                    for n in range(2):
                        fillers.append((None, lambda m=m, n=n: phase3_mn(m, n)))

            import concourse.bass as bass_mod

            def normalize(t, jq, evicted):
                # evicted: [(ouz_h0, ...), (ouz_h1, ...)] for pair t at chunk jq.
                # Pack both heads' Z rows [1,512] as [8,64] each -> one [16,64]
                # reciprocal (64 elems/lane), then broadcast 1/(128 Z) via a
                # DRAM round-trip (partition-step-0 DMA reads are legal from
                # DRAM).  The x1/128 undoes the v weight pre-scale.
                qs2 = slice(jq * 512, (jq + 1) * 512)
                zb = znp.tile([16, 64], F32, tag="zb", bufs=2, name=f"zb{t}_{jq}")
                for hh in range(2):
                    ouz = evicted[hh]
                    nc.sync.dma_start(
                        zb[8 * hh:8 * hh + 8, :],
                        ouz[64:65, :].rearrange("o (p q) -> o p q", p=8),
                    )
                rcp = znp.tile([16, 64], F32, tag="rcpb", bufs=2, name=f"rcp{t}_{jq}")
                nc.vector.reciprocal(rcp, zb)
                rcp16 = znp.tile([16, 64], BT16, tag="rcp16b", bufs=2, name=f"rcp16{t}_{jq}")
                nc.vector.tensor_scalar_mul(rcp16, rcp, 1.0 / WS)
                for hh in range(2):
                    nc.sync.dma_start(
                        rcp_dram[jq, 2 * t + hh, :].rearrange("(p q) -> p q", p=8),
                        rcp16[8 * hh:8 * hh + 8, :],
                    )
                tmps = []
                for hh in range(2):
                    ouz = evicted[hh]
                    bc_sb = znp.tile([64, 512], BT16, tag="bc_sb", bufs=3, name=f"bs{t}_{jq}_{hh}")
                    src = rcp_dram[jq, 2 * t + hh, :]
                    bcast = bass_mod.AP(
                        tensor=src.tensor, offset=src.offset,
                        ap=[[0, 64]] + [list(a) for a in src.ap],
                    )
                    nc.sync.dma_start(bc_sb, bcast)
                    tmps.append((ouz, bc_sb))
                for hh, (ouz, bc_sb) in enumerate(tmps):
                    if hh == 0:
                        nc.vector.tensor_mul(oT[t][0:64, qs2], ouz[0:64, :], bc_sb)
                    else:
                        tmp = znp.tile([64, 512], BT16, tag="tmp_o", bufs=2, name=f"tm{t}_{jq}")
                        nc.vector.tensor_mul(tmp, ouz[0:64, :], bc_sb)
                        nc.gpsimd.dma_start(oT[t][64:128, qs2], tmp)

            def normalize_tail(t, jq, evicted):
                # Last pair of the last chunk: the Z broadcast sits on the
                # kernel's critical tail, so skip the DRAM round-trip and
                # broadcast 1/(128 Z) via a K=1 outer product on the PE.
                rcps = []
                for hh in range(2):
                    ouz = evicted[hh]
                    zrow = znp.tile([1, 512], BT16, tag="zrow", bufs=2, name=f"zr{hh}")
                    nc.gpsimd.tensor_copy(zrow, ouz[64:65, :])
                    zbc = ps.tile([64, 512], F32, tag="ot", bufs=4, name=f"zbc{hh}")
                    nc.tensor.matmul(zbc, bconst, zrow, start=True, stop=True)
                    rcp_bc = znp.tile([64, 512], BT16, tag="rcpbc", bufs=2, name=f"rb{hh}")
                    with nc.allow_low_precision(reason="1/Z broadcast in bf16; 0.4% rel err on one tail pair"):
                        nc.vector.reciprocal(rcp_bc, zbc)
                    rcps.append(rcp_bc)
                # per-128-col muls; each phase3 m-chunk follows immediately
                # after its slice is ready
                for mi in range(4):
                    cs = slice(mi * 128, (mi + 1) * 128)
                    gs = slice(jq * 512 + mi * 128, jq * 512 + (mi + 1) * 128)
                    for hh in range(2):
                        if hh == 0:
                            nc.vector.tensor_mul(
                                oT[t][0:64, gs], evicted[0][0:64, cs], rcps[0][:, cs]
                            )
                        else:
                            tmp = znp.tile([64, 128], BT16, tag="tmp_os", bufs=4, name=f"tms{mi}")
                            nc.vector.tensor_mul(tmp, evicted[1][0:64, cs], rcps[1][:, cs])
                            nc.sync.dma_start(oT[t][64:128, gs], tmp)
                    phase3_m(4 * jq + mi)

            phase1(0)
            if NJQ > 1:
                phase1(1, defer=True)
            pend = []          # (t, jq, evicted) not yet normalized
            for jq in range(NJQ):
                drain_for(jq)  # phase1(jq) granules must all be emitted
                for t in range(NP):
                    ev = attention(t, jq)
                    if pend:
                        pt_, pjq_, pev_ = pend.pop(0)
                        normalize(pt_, pjq_, pev_)
                        if pt_ == NP - 1:
                            phase3(pjq_)   # queues fillers
                    pend.append((t, jq, ev))
                if jq + 2 < NJQ:
                    phase1(jq + 2, defer=True)
            drain(len(fillers))
            # tail: normalize remaining entries; the last one interleaves
            # its normalization with phase3 m-chunks via the PE broadcast
            for pt_, pjq_, pev_ in pend[:-1]:
                normalize(pt_, pjq_, pev_)
                if pt_ == NP - 1:
                    phase3(pjq_)
                    drain(len(fillers))
            pt_, pjq_, pev_ = pend[-1]
            normalize_tail(pt_, pjq_, pev_)

    nc.compile()
    return nc


def _host_prep(x, wq, bq, wk, bk, wv, wo):
    qn = np.arange(128)[None, :]
    kn = np.arange(128)[:, None]
    masks_np = (qn >= kn).astype(BF16)

    per_g = []
    for g in range(G):
        cs = slice(g * HD, (g + 1) * HD)
        per_g.append({
            "wq": np.ascontiguousarray(wq[:, cs] * WS).astype(FP8),
            "wk": np.ascontiguousarray(wk[:, cs] * WS).astype(FP8),
            "wv": np.ascontiguousarray(wv[:, cs] * WS).astype(FP8),
            "wq16": np.ascontiguousarray(wq[:, cs]).astype(BF16),
            "wk16": np.ascontiguousarray(wk[:, cs]).astype(BF16),
            "wv16": np.ascontiguousarray(wv[:, cs]).astype(BF16),
            "wo": np.ascontiguousarray(wo[cs, :]).astype(BF16),
            "bq": np.ascontiguousarray((bq[cs] / 8.0).reshape(NP, 128).T).astype(np.float32),
            "bk": np.ascontiguousarray(bk[cs].reshape(NP, 128).T).astype(np.float32),
            "masks": masks_np,
        })
    in_maps = []
    for c in range(8):
        b, g = divmod(c, G)
        m = dict(per_g[g])
        xt_full = np.ascontiguousarray(x[b].T)
        m["xT"] = xt_full.astype(FP8)
        m["xT16"] = np.ascontiguousarray(xt_full[:, 0:512]).astype(BF16)
        in_maps.append(m)
    return in_maps


def kernel(x, wq, bq, wk, bk, wv, bv, wo, bo):
    x = np.asarray(x, dtype=np.float32)
    wq = np.asarray(wq, dtype=np.float32)
    bq = np.asarray(bq, dtype=np.float32)
    wk = np.asarray(wk, dtype=np.float32)
    bk = np.asarray(bk, dtype=np.float32)
    wv = np.asarray(wv, dtype=np.float32)
    bv = np.asarray(bv, dtype=np.float32)
    wo = np.asarray(wo, dtype=np.float32)
    bo = np.asarray(bo, dtype=np.float32)

    if "nc" not in _CACHED:
        _CACHED["nc"] = _build()
    nc = _CACHED["nc"]

    in_maps = _host_prep(x, wq, bq, wk, bk, wv, wo)
    res = run_bass_kernel_spmd(nc, in_maps, core_ids=list(range(8)))

    const_row = (bo.astype(np.float64) + bv.astype(np.float64) @ wo.astype(np.float64))
    out = np.empty((B, T, C), dtype=np.float32)
    for b in range(B):
        acc = res.results[2 * b]["y"].astype(np.float64)
        acc += res.results[2 * b + 1]["y"]
        acc += const_row[None, :]
        out[b] = acc.astype(np.float32)
    return out


# revision 22
# speedup vs baseline: 1.4188x; 1.0107x over previous
"""Causal self-attention (B=4, T=2048, C=1024, H=16, D=64) on 8 trn2 NeuronCores.

Sharding: core c = (batch b = c//2, head-group g = c%2). Megatron-style within a
batch: each core computes 8 heads' q/k/v (column-parallel) and a row-parallel
partial out-projection. Host sums the two partials per batch and adds the
rank-1 bias term (bo + bv @ wo) -- valid because softmax rows sum to 1, so v's
bias never needs to enter the kernel.

Per-core kernel:
  phase 1 (per 512-wide T chunk): qT,kT = (x@w)^T.  Chunks 1-3 use fp8e4
           DoubleRow matmuls (weights host-prescaled x128 to clear e4m3
           subnormals; x in fp8e4); chunk 0 runs in bf16 because early
           attention rows (tiny softmax support) get no error averaging and
           kT tiles 0-3 feed every q-chunk.  q/k/v all stay x128-scaled in
           SBUF -- the descale is folded into exp's free scale immediate
           (0.125/128^2) and the 1/Z reciprocal (x 1/128), so evictions are
           plain add/copy ops.  v gets a ones column appended per head.
  phase 2: flash-style streaming attention in S^T orientation:
           S^T[k,q] = kT.T @ qT bf16 (head pairs in PE row groups 0/64 run
           concurrently via row tiling), P^T = exp(S^T * 2^-17) on ScalarE;
           causal masking via an in-place [128,128] tril multiply on only
           the diagonal band; O^T accumulated via lhsT=v_tile (stationary),
           rhs=P^T; the ones column of v makes PSUM row 64 the softmax
           denominator Z for free.  1/(128 Z) broadcast across partitions
           with a DRAM round-trip (partition-step-0 DMA reads are legal
           from DRAM).
  phase 3 (per T chunk, overlapped with the next chunk's attention):
           y = O @ wo bf16 via lhsT=O^T (already the natural layout).
"""
import numpy as np
import ml_dtypes

import concourse.tile as tile
from concourse import bacc, mybir
from concourse.bass_utils import run_bass_kernel_spmd

BF16 = ml_dtypes.bfloat16
FP8 = ml_dtypes.float8_e4m3
F32 = mybir.dt.float32
BT16 = mybir.dt.bfloat16
F8E4 = mybir.dt.float8e4
AF = mybir.ActivationFunctionType
ALU = mybir.AluOpType
DR = mybir.MatmulPerfMode.DoubleRow

B, T, C, H, D = 4, 2048, 1024, 16, 64
G = 2              # head groups (cores per batch)
HL = H // G        # heads per core = 8
HD = HL * D        # local head dims = 512
NP = 4             # head pairs per core
NJQ = T // 512     # q chunks of 512 = 4
NIK = T // 128     # k tiles of 128 = 16
KC = C // 128      # contraction chunks = 8
WS = 128.0         # host pre-scale on fp8 weights
SC = 0.125 / (WS * WS)   # exp scale: 1/sqrt(D) and the two x128 descales

_CACHED = {}


def _build():
    nc = bacc.Bacc("TRN2", debug=False)
    xT = nc.dram_tensor("xT", [C, T], F8E4, kind="ExternalInput").ap()
    xT16 = nc.dram_tensor("xT16", [C, 512], BT16, kind="ExternalInput").ap()
    wq = nc.dram_tensor("wq", [C, HD], F8E4, kind="ExternalInput").ap()
    wk = nc.dram_tensor("wk", [C, HD], F8E4, kind="ExternalInput").ap()
    wv = nc.dram_tensor("wv", [C, HD], F8E4, kind="ExternalInput").ap()
    wq16 = nc.dram_tensor("wq16", [C, HD], BT16, kind="ExternalInput").ap()
    wk16 = nc.dram_tensor("wk16", [C, HD], BT16, kind="ExternalInput").ap()
    wv16 = nc.dram_tensor("wv16", [C, HD], BT16, kind="ExternalInput").ap()
    wo = nc.dram_tensor("wo", [HD, C], BT16, kind="ExternalInput").ap()
    bq = nc.dram_tensor("bq", [128, NP], F32, kind="ExternalInput").ap()
    bk = nc.dram_tensor("bk", [128, NP], F32, kind="ExternalInput").ap()
    masks = nc.dram_tensor("masks", [128, 128], BT16, kind="ExternalInput").ap()
    rcp_dram = nc.dram_tensor("rcp_dram", [NJQ, 8, 512], BT16).ap()
    y = nc.dram_tensor("y", [T, C], F32, kind="ExternalOutput").ap()

    with tile.TileContext(nc) as tc:
        with (
            tc.tile_pool(name="consts", bufs=1) as consts,
            tc.tile_pool(name="xt", bufs=3) as xtp,
            tc.tile_pool(name="qk", bufs=1) as qkp,
            tc.tile_pool(name="vp", bufs=1) as vp,
            tc.tile_pool(name="otp", bufs=1) as otp,
            tc.tile_pool(name="pt", bufs=6) as ptp,
            tc.tile_pool(name="zn", bufs=3) as znp,
            tc.tile_pool(name="yst", bufs=4) as ystp,
            tc.tile_pool(name="ps", bufs=2, space="PSUM") as ps,
        ):
            # ---- constants (biases are tiny and gate evictions: load them first) ----
            bq_dma = consts.tile([128, NP], F32, tag="bq_dma")
            bq_sb = consts.tile([128, NP], F32, tag="bq")
            nc.sync.dma_start(bq_dma, bq)
            nc.vector.tensor_copy(bq_sb, bq_dma)
            bk_dma = consts.tile([128, NP], F32, tag="bk_dma")
            bk_sb = consts.tile([128, NP], F32, tag="bk")
            nc.sync.dma_start(bk_dma, bk)
            nc.vector.tensor_copy(bk_sb, bk_dma)
            # bf16 weights + chunk-0 x first: phase1(0) runs in bf16
            wq16_sb = consts.tile([128, KC, HD], BT16, tag="wq16")
            wq16_r = wq16.rearrange("(k p) c -> p k c", p=128)
            xt0 = xtp.tile([128, KC, 512], BT16, tag="xt16", name="xt_pre0")
            x0_r = xT16.rearrange("(k p) t -> p k t", p=128)
            masks_dma = consts.tile([128, 128], BT16, tag="masks_dma")
            masks_sb = consts.tile([128, 128], BT16, tag="masks")
            nc.gpsimd.dma_start(masks_dma, masks)
            nc.gpsimd.tensor_copy(masks_sb, masks_dma)
            wk16_sb = consts.tile([128, KC, HD], BT16, tag="wk16")
            wk16_r = wk16.rearrange("(k p) c -> p k c", p=128)
            for k in range(KC):
                nc.scalar.dma_start(wq16_sb[:, k, :], wq16_r[:, k, :])
                nc.scalar.dma_start(wk16_sb[:, k, :], wk16_r[:, k, :])
                (nc.sync if k < 4 else nc.gpsimd).dma_start(xt0[:, k, :], x0_r[:, k, :])
            wv16_sb = consts.tile([128, KC, HD], BT16, tag="wv16")
            nc.gpsimd.dma_start(wv16_sb, wv16.rearrange("(k p) c -> p k c", p=128))
            wq_sb = consts.tile([128, KC, HD], F8E4, tag="wq")
            nc.scalar.dma_start(wq_sb, wq.rearrange("(k p) c -> p k c", p=128))
            wk_sb = consts.tile([128, KC, HD], F8E4, tag="wk")
            nc.sync.dma_start(wk_sb, wk.rearrange("(k p) c -> p k c", p=128))
            wv_sb = consts.tile([128, KC, HD], F8E4, tag="wv")
            nc.gpsimd.dma_start(wv_sb, wv.rearrange("(k p) c -> p k c", p=128))
            wo_sb = consts.tile([128, NP, C], BT16, tag="wo")
            nc.gpsimd.dma_start(wo_sb, wo.rearrange("(t p) c -> p t c", p=128))
            bconst = consts.tile([1, 64], BT16, tag="bconst")
            nc.vector.memset(bconst, WS)
            # ---- persistent activations ----
            qT = [qkp.tile([128, T], BT16, tag=f"qT{t}", name=f"qT{t}") for t in range(NP)]
            kT = [qkp.tile([128, T], BT16, tag=f"kT{t}", name=f"kT{t}") for t in range(NP)]
            v_sb = [vp.tile([128, HL * 65], BT16, tag=f"v{i}", name=f"v{i}") for i in range(NIK)]
            oT = [otp.tile([128, T], BT16, tag=f"oT{t}", name=f"oT{t}") for t in range(NP)]

            from collections import deque
            fillers = deque()   # (jt_deadline, closure) granules drained
                                # one-per-attention-tile so phase1/phase3
                                # matmuls interleave finely with the
                                # exp-paced attention stream

            def drain(n=1):
                for _ in range(min(n, len(fillers))):
                    fillers.popleft()[1]()

            def drain_for(jt):
                while any(d is not None and d <= jt for d, _ in fillers):
                    fillers.popleft()[1]()

            def phase1(jt, defer=False):
                # jt==0 runs in bf16: early attention rows (small softmax
                # support) get no error averaging, and kT tiles 0-3 feed
                # every q-chunk.  q/k/v leave this phase x128-scaled.
                bf = (jt == 0)
                if bf:
                    xt = xt0
                else:
                    xt = xtp.tile([128, KC, 512], F8E4, tag="xt", name=f"xt{jt}")
                    xr = xT[:, jt * 512:(jt + 1) * 512].rearrange("(k p) t -> p k t", p=128)
                    for k in range(KC):
                        (nc.sync if k % 2 == 0 else nc.scalar).dma_start(xt[:, k, :], xr[:, k, :])
                wqs, wks, wvs = (
                    (wq16_sb, wk16_sb, wv16_sb) if bf else (wq_sb, wk_sb, wv_sb)
                )

                def mm_acc(p, w_sb, tsl):
                    if bf:
                        for k in range(KC):
                            nc.tensor.matmul(
                                p, w_sb[:, k, tsl], xt[:, k, :],
                                start=(k == 0), stop=(k == KC - 1),
                            )
                    else:
                        for k in range(0, KC, 2):
                            nc.tensor.matmul(
                                p, w_sb[:, k:k + 2, tsl], xt[:, k:k + 2, :],
                                start=(k == 0), stop=(k == KC - 2), perf_mode=DR,
                            )

                ptag = "st" if bf else "ot"

                def q_gran(t):
                    p = ps.tile([128, 512], F32, tag=ptag, bufs=(2 if bf else 4), name=f"pq{jt}_{t}")
                    mm_acc(p, wqs, slice(t * 128, (t + 1) * 128))
                    nc.vector.tensor_scalar(
                        qT[t][:, jt * 512:(jt + 1) * 512], p,
                        0.125 if bf else 0.125 / WS, bq_sb[:, t:t + 1],
                        ALU.mult, ALU.add,
                    )

                def k_gran(t):
                    p = ps.tile([128, 512], F32, tag=ptag, bufs=(2 if bf else 4), name=f"pk{jt}_{t}")
                    mm_acc(p, wks, slice(t * 128, (t + 1) * 128))
                    if bf:
                        nc.vector.tensor_scalar_add(
                            kT[t][:, jt * 512:(jt + 1) * 512], p, bk_sb[:, t:t + 1]
                        )
                    else:
                        nc.vector.tensor_scalar(
                            kT[t][:, jt * 512:(jt + 1) * 512], p,
                            1.0 / WS, bk_sb[:, t:t + 1], ALU.mult, ALU.add,
                        )

                def v_gran(s):
                    ik = jt * 4 + s
                    p = ps.tile([128, 512], F32, tag=ptag, bufs=(2 if bf else 4), name=f"pv{ik}")
                    if bf:
                        for k in range(KC):
                            nc.tensor.matmul(
                                p, xt[:, k, s * 128:(s + 1) * 128], wvs[:, k, :],
                                start=(k == 0), stop=(k == KC - 1),
                            )
                    else:
                        for k in range(0, KC, 2):
                            nc.tensor.matmul(
                                p, xt[:, k:k + 2, s * 128:(s + 1) * 128], wvs[:, k:k + 2, :],
                                start=(k == 0), stop=(k == KC - 2), perf_mode=DR,
                            )
                    vg = v_sb[ik].rearrange("p (h c) -> p h c", c=65)
                    pg = p.rearrange("p (h c) -> p h c", c=64)
                    if bf:
                        nc.vector.tensor_scalar_mul(vg[:, :, 0:64], pg, WS)
                    else:
                        nc.vector.tensor_copy(vg[:, :, 0:64], pg)
                    nc.vector.memset(vg[:, :, 64:65], 1.0)

                grans = (
                    [(lambda t=t: q_gran(t)) for t in range(NP)]
                    + [(lambda t=t: k_gran(t)) for t in range(NP)]
                    + [(lambda s=s: v_gran(s)) for s in range(4)]
                )
                if defer:
                    for g in grans:
                        fillers.append((jt, g))
                else:
                    for g in grans:
                        g()

            def av(t, ik, nik, pts, o_ps):
                pt, c0 = pts[ik]
                ptg = pt.rearrange("p (h q) -> p h q", q=512)
                for hh in range(2):
                    h = 2 * t + hh
                    nc.tensor.matmul(
                        o_ps[hh][:, c0:512], v_sb[ik][:, h * 65:h * 65 + 65],
                        ptg[:, hh, c0:512],
                        start=(ik == 0), stop=(ik == nik - 1),
                    )

            def attention(t, jq, tail=False):
                nik = 4 * jq + 4
                o_ps = [
                    ps.tile([65, 512], F32, tag="ot", bufs=4, name=f"ops{t}_{jq}_{_h}")
                    for _h in range(2)
                ]
                pts = {}
                for ik in range(nik):
                    d = ik - 4 * jq
                    c0 = 128 * d if d > 0 else 0   # first potentially-valid column
                    st = ps.tile([128, 1024], F32, tag="st", name=f"st{t}_{jq}_{ik}")
                    stg = st.rearrange("p (h q) -> p h q", q=512)
                    for hh in range(2):
                        r = slice(hh * 64, hh * 64 + 64)
                        nc.tensor.matmul(
                            stg[:, hh, c0:512],
                            kT[t][r, ik * 128:(ik + 1) * 128],
                            qT[t][r, jq * 512 + c0:(jq + 1) * 512],
                            start=True, stop=True,
                        )
                    pt = ptp.tile([128, 1024], BT16, tag="pt", name=f"pt{t}_{jq}_{ik}")
                    ptg = pt.rearrange("p (h q) -> p h q", q=512)
                    if d >= 0:
                        nc.scalar.activation(ptg[:, :, c0:512], stg[:, :, c0:512], AF.Exp)
                        # in-place tril mask on just the 128-wide diagonal band
                        for hh in range(2):
                            nc.vector.tensor_mul(
                                ptg[:, hh, c0:c0 + 128],
                                ptg[:, hh, c0:c0 + 128], masks_sb,
                            )
                    else:
                        nc.scalar.activation(pt, st, AF.Exp)
                    pts[ik] = (pt, c0)
                    if ik > 0:
                        av(t, ik - 1, nik, pts, o_ps)
                    drain(1)
                av(t, nik - 1, nik, pts, o_ps)
                rcps = []
                if tail:
                    # Z rows are final: extract + broadcast 1/(128 Z) now,
                    # while ScalarE is idle, so the tail normalize only has
                    # the muls and phase3 left on its critical path.
                    for hh in range(2):
                        zrow = znp.tile([1, 512], BT16, tag="zrow", bufs=2, name=f"zr{hh}")
                        (nc.scalar.copy if hh == 0 else nc.vector.tensor_copy)(
                            zrow, o_ps[hh][64:65, :]
                        )
                        zbc = ps.tile([64, 512], F32, tag="ot", bufs=4, name=f"zbc{hh}")
                        nc.tensor.matmul(zbc, bconst, zrow, start=True, stop=True)
                        rcp_bc = znp.tile([64, 512], BT16, tag="rcpbc", bufs=2, name=f"rb{hh}")
                        with nc.allow_low_precision(reason="1/Z in bf16; 0.4% on one pair"):
                            nc.vector.reciprocal(rcp_bc, zbc)
                        rcps.append(rcp_bc)
                # evict Z row + unnormalized O^T, freeing the PSUM accumulators
                out_h = []
                for hh in range(2):
                    ouz = znp.tile([65, 512], F32, tag="ouz", bufs=6, name=f"oz{t}_{jq}_{hh}")
                    nc.vector.tensor_copy(ouz, o_ps[hh])
                    out_h.append(ouz)
                return out_h, rcps

            def phase3_mn(m, n, tag="ot"):
                p = ps.tile([128, 512], F32, tag=tag, bufs=(2 if tag == "st" else 4), name=f"py{m}_{n}")
                for t in range(NP):
                    nc.tensor.matmul(
                        p, oT[t][:, m * 128:(m + 1) * 128],
                        wo_sb[:, t, n * 512:(n + 1) * 512],
                        start=(t == 0), stop=(t == NP - 1),
                    )
                ys = ystp.tile([128, 512], F32, tag="y", name=f"ys{m}_{n}")
                nc.vector.tensor_copy(ys, p)
                nc.gpsimd.dma_start(
                    y[m * 128:(m + 1) * 128, n * 512:(n + 1) * 512], ys
                )

            def phase3_m(m):
                for n in range(2):
                    phase3_mn(m, n, tag="st")

            def phase3(jq):
                for m in range(4 * jq, 4 * jq + 4):
                    for n in range(2):
                        fillers.append((None, lambda m=m, n=n: phase3_mn(m, n)))

            import concourse.bass as bass_mod

            def normalize(t, jq, evicted):
                # evicted: [(ouz_h0, ...), (ouz_h1, ...)] for pair t at chunk jq.
                # Pack both heads' Z rows [1,512] as [8,64] each -> one [16,64]
                # reciprocal (64 elems/lane), then broadcast 1/(128 Z) via a
                # DRAM round-trip (partition-step-0 DMA reads are legal from
                # DRAM).  The x1/128 undoes the v weight pre-scale.
                qs2 = slice(jq * 512, (jq + 1) * 512)
                zb = znp.tile([16, 64], F32, tag="zb", bufs=2, name=f"zb{t}_{jq}")
                for hh in range(2):
                    ouz = evicted[hh]
                    nc.sync.dma_start(
                        zb[8 * hh:8 * hh + 8, :],
                        ouz[64:65, :].rearrange("o (p q) -> o p q", p=8),
                    )
                rcp = znp.tile([16, 64], F32, tag="rcpb", bufs=2, name=f"rcp{t}_{jq}")
                nc.vector.reciprocal(rcp, zb)
                rcp16 = znp.tile([16, 64], BT16, tag="rcp16b", bufs=2, name=f"rcp16{t}_{jq}")
                nc.vector.tensor_scalar_mul(rcp16, rcp, 1.0 / WS)
                for hh in range(2):
                    nc.sync.dma_start(
                        rcp_dram[jq, 2 * t + hh, :].rearrange("(p q) -> p q", p=8),
                        rcp16[8 * hh:8 * hh + 8, :],
                    )
                tmps = []
                for hh in range(2):
                    ouz = evicted[hh]
                    bc_sb = znp.tile([64, 512], BT16, tag="bc_sb", bufs=3, name=f"bs{t}_{jq}_{hh}")
                    src = rcp_dram[jq, 2 * t + hh, :]
                    bcast = bass_mod.AP(
                        tensor=src.tensor, offset=src.offset,
                        ap=[[0, 64]] + [list(a) for a in src.ap],
                    )
                    nc.sync.dma_start(bc_sb, bcast)
                    tmps.append((ouz, bc_sb))
                for hh, (ouz, bc_sb) in enumerate(tmps):
                    if hh == 0:
                        nc.vector.tensor_mul(oT[t][0:64, qs2], ouz[0:64, :], bc_sb)
                    else:
                        tmp = znp.tile([64, 512], BT16, tag="tmp_o", bufs=2, name=f"tm{t}_{jq}")
                        nc.vector.tensor_mul(tmp, ouz[0:64, :], bc_sb)
                        nc.gpsimd.dma_start(oT[t][64:128, qs2], tmp)

            def normalize_tail(t, jq, evicted, rcps):
                # Last pair of the last chunk: 1/(128 Z) was already
                # broadcast via the PE inside the final attention call.
                # per-128-col muls; each phase3 m-chunk follows immediately
                # after its slice is ready
                for mi in range(4):
                    cs = slice(mi * 128, (mi + 1) * 128)
                    gs = slice(jq * 512 + mi * 128, jq * 512 + (mi + 1) * 128)
                    for hh in range(2):
                        if hh == 0:
                            nc.vector.tensor_mul(
                                oT[t][0:64, gs], evicted[0][0:64, cs], rcps[0][:, cs]
                            )
                        else:
                            tmp = znp.tile([64, 128], BT16, tag="tmp_os", bufs=4, name=f"tms{mi}")
                            nc.vector.tensor_mul(tmp, evicted[1][0:64, cs], rcps[1][:, cs])
                            nc.sync.dma_start(oT[t][64:128, gs], tmp)
                    phase3_m(4 * jq + mi)

            phase1(0)
            if NJQ > 1:
                phase1(1, defer=True)
            pend = []          # (t, jq, evicted) not yet normalized
            for jq in range(NJQ):
                drain_for(jq)  # phase1(jq) granules must all be emitted
                for t in range(NP):
                    ev, tail_rcps = attention(
                        t, jq, tail=(jq == NJQ - 1 and t == NP - 1)
                    )
                    if pend:
                        pt_, pjq_, pev_ = pend.pop(0)
                        normalize(pt_, pjq_, pev_)
                        if pt_ == NP - 1:
                            phase3(pjq_)   # queues fillers
                    pend.append((t, jq, ev))
                    if jq == NJQ - 1 and t == NP - 1:
                        last_rcps = tail_rcps
                if jq + 2 < NJQ:
                    phase1(jq + 2, defer=True)
            drain(len(fillers))
            # tail: normalize remaining entries; the last one interleaves
            # its normalization with phase3 m-chunks via the PE broadcast
            for pt_, pjq_, pev_ in pend[:-1]:
                normalize(pt_, pjq_, pev_)
                if pt_ == NP - 1:
                    phase3(pjq_)
                    drain(len(fillers))
            pt_, pjq_, pev_ = pend[-1]
            normalize_tail(pt_, pjq_, pev_, last_rcps)

    nc.compile()
    return nc


def _host_prep(x, wq, bq, wk, bk, wv, wo):
    qn = np.arange(128)[None, :]
    kn = np.arange(128)[:, None]
    masks_np = (qn >= kn).astype(BF16)

    per_g = []
    for g in range(G):
        cs = slice(g * HD, (g + 1) * HD)
        per_g.append({
            "wq": np.ascontiguousarray(wq[:, cs] * WS).astype(FP8),
            "wk": np.ascontiguousarray(wk[:, cs] * WS).astype(FP8),
            "wv": np.ascontiguousarray(wv[:, cs] * WS).astype(FP8),
            "wq16": np.ascontiguousarray(wq[:, cs]).astype(BF16),
            "wk16": np.ascontiguousarray(wk[:, cs]).astype(BF16),
            "wv16": np.ascontiguousarray(wv[:, cs]).astype(BF16),
            "wo": np.ascontiguousarray(wo[cs, :]).astype(BF16),
            "bq": np.ascontiguousarray((bq[cs] / 8.0).reshape(NP, 128).T).astype(np.float32),
            "bk": np.ascontiguousarray(bk[cs].reshape(NP, 128).T).astype(np.float32),
            "masks": masks_np,
        })
    in_maps = []
    for c in range(8):
        b, g = divmod(c, G)
        m = dict(per_g[g])
        xt_full = np.ascontiguousarray(x[b].T)
        m["xT"] = xt_full.astype(FP8)
        m["xT16"] = np.ascontiguousarray(xt_full[:, 0:512]).astype(BF16)
        in_maps.append(m)
    return in_maps


def kernel(x, wq, bq, wk, bk, wv, bv, wo, bo):
    x = np.asarray(x, dtype=np.float32)
    wq = np.asarray(wq, dtype=np.float32)
    bq = np.asarray(bq, dtype=np.float32)
    wk = np.asarray(wk, dtype=np.float32)
    bk = np.asarray(bk, dtype=np.float32)
    wv = np.asarray(wv, dtype=np.float32)
    bv = np.asarray(bv, dtype=np.float32)
    wo = np.asarray(wo, dtype=np.float32)
    bo = np.asarray(bo, dtype=np.float32)

    if "nc" not in _CACHED:
        _CACHED["nc"] = _build()
    nc = _CACHED["nc"]

    in_maps = _host_prep(x, wq, bq, wk, bk, wv, wo)
    res = run_bass_kernel_spmd(nc, in_maps, core_ids=list(range(8)))

    const_row = (bo.astype(np.float64) + bv.astype(np.float64) @ wo.astype(np.float64))
    out = np.empty((B, T, C), dtype=np.float32)
    for b in range(B):
        acc = res.results[2 * b]["y"].astype(np.float64)
        acc += res.results[2 * b + 1]["y"]
        acc += const_row[None, :]
        out[b] = acc.astype(np.float32)
    return out
